# revision 35
# baseline (speedup 1.0000x reference)
"""Trainium2 Bass kernel for nn_ConvolutionalCapsules.

Sharding: core c (of 8) owns output-capsules {2*(c%4), 2*(c%4)+1} for batches
{2*(c//4), 2*(c//4)+1}. Each core runs the p4 group conv with 128 output
channels (2 nout x 16 dout x 4 rot) over its 16 images (2 batches x 8 input
capsules), then LayerNorm + degree-score routing + squash per (batch, nout).

Conv: 3x3 conv as shifted matmuls from a zero-padded fp16 SBUF image (34x34
rows, dual-copy: partitions 0-63 = padded image, 64-127 = same shifted one row
so one K=128 matmul covers two filter taps). Weights are the stationary
operand ([K, 128 out-channels]), so each of the 6 matmuls per 512 positions
retires 2x the baseline's work. PE then transposes u ([128 chan, pos]) to the
routing layout ([128 pos, chan]) in fp16 (4x cheaper than f32 transposes).

Routing runs fully in fp16 on the DVE (tensor_tensor at the 2x perf mode,
tensor_scalar at 4x); segmented reductions over d/i/g are binary tree-adds of
strided views, which beat TensorReduce ~4x. LayerNorm is algebraically folded:
up = (T - mu)*rstd, rr_i = (up_i . S) * (var+eps)/(16 var), softmax over i
(shift-free: |rr| <= 8), s = sum_i score_i up_i, squash over g.

Host packs inputs (pad + dual-copy + fp16) and unpacks the [pos, (b,n,k,d,g)]
fp16 output, so every DMA moves contiguous >=512B lines.
"""

import numpy as np
from contextlib import ExitStack

import concourse.bass as bass
import concourse.tile as tile
from concourse import mybir
from concourse.bass_utils import run_bass_kernel_spmd

F16 = mybir.dt.float16
F32 = mybir.dt.float32
AF = mybir.ActivationFunctionType
OP = mybir.AluOpType

_ENGINES = {
    mybir.EngineType.PE,
    mybir.EngineType.Activation,
    mybir.EngineType.Pool,
    mybir.EngineType.DVE,
    mybir.EngineType.SP,
}


def _split_sync_waits(nc):
    """This walrus build accepts a single embedded sync-wait per instruction;
    hoist extras onto preceding NoOps on the same engine (ge-imm waits commute)."""
    for f in nc.m.functions:
        for bb in f.blocks:
            newl = []
            changed = False
            for inst in list(bb.instructions):
                si = inst.sync_info
                waits = list(si.on_wait) if si and si.on_wait else []
                if len(waits) > 1 and inst.engine in _ENGINES:
                    changed = True
                    for k, w in enumerate(waits[:-1]):
                        newl.append(
                            mybir.InstNoOp(
                                name=f"{inst.name}-ws{k}",
                                ins=[],
                                outs=[],
                                engine=inst.engine,
                                sync_info=mybir.SyncInfo(on_wait=[w], on_update=[]),
                            )
                        )
                    si.on_wait = waits[-1:]
                    inst.sync_info = si
                newl.append(inst)
            if changed:
                bb.instructions = newl


def build_program(apply_bias=False, apply_gb=False):
    nc = bass.Bass(trn_type="TRN2")
    capsd = nc.dram_tensor("capsd", [16, 128, 1164], F16, kind="ExternalInput")
    w = nc.dram_tensor("w", [128, 768], F16, kind="ExternalInput")
    ident = nc.dram_tensor("ident", [128, 128], F16, kind="ExternalInput")
    mmu = nc.dram_tensor("mmu", [128, 16], F16, kind="ExternalInput")
    if apply_bias:
        cb = nc.dram_tensor("cb", [1, 128], F16, kind="ExternalInput")
    if apply_gb:
        gam = nc.dram_tensor("gam", [1, 16], F16, kind="ExternalInput")
        bet = nc.dram_tensor("bet", [1, 16], F16, kind="ExternalInput")
    outd = nc.dram_tensor("outd", [128, 2048], F16, kind="ExternalOutput")

    with tile.TileContext(nc) as tc:
        with nc.allow_low_precision(reason="fp16 routing; 2e-2 rel tolerance"), \
             ExitStack() as ctx:
            consts = ctx.enter_context(tc.tile_pool(name="consts", bufs=1))
            imgs = ctx.enter_context(tc.tile_pool(name="imgs", bufs=2))
            us = ctx.enter_context(tc.tile_pool(name="us", bufs=1))
            ps = ctx.enter_context(tc.tile_pool(name="ps", bufs=4, space="PSUM"))
            tps = ctx.enter_context(tc.tile_pool(name="tps", bufs=3, space="PSUM"))
            sps = ctx.enter_context(tc.tile_pool(name="sps", bufs=1, space="PSUM"))
            Tp = ctx.enter_context(tc.tile_pool(name="Tp", bufs=1))
            qp = ctx.enter_context(tc.tile_pool(name="qp", bufs=1))
            scr = ctx.enter_context(tc.tile_pool(name="scr", bufs=2))
            trees = ctx.enter_context(tc.tile_pool(name="trees", bufs=1))
            sm = ctx.enter_context(tc.tile_pool(name="sm", bufs=1))
            vp = ctx.enter_context(tc.tile_pool(name="vp", bufs=2))

            w_sb = consts.tile([128, 768], F16, tag="w")
            nc.sync.dma_start(out=w_sb[:], in_=w.ap())
            id_sb = consts.tile([128, 128], F16, tag="ident")
            nc.sync.dma_start(out=id_sb[:], in_=ident.ap())
            mmu_sb = consts.tile([128, 16], F16, tag="mmu")
            nc.sync.dma_start(out=mmu_sb[:], in_=mmu.ap())
            eps5 = consts.tile([128, 1], F32, tag="eps5")
            nc.vector.memset(eps5[:], 1e-5)
            eps16 = consts.tile([128, 1], F32, tag="eps16")
            nc.vector.memset(eps16[:], 1e-16)
            if apply_bias:
                cb_sb = consts.tile([1, 128], F16, tag="cb")
                nc.sync.dma_start(out=cb_sb[:], in_=cb.ap())
                ones512 = consts.tile([1, 512], F16, tag="ones512")
                nc.vector.memset(ones512[:], 1.0)
            if apply_gb:
                gam_sb = consts.tile([128, 16], F16, tag="gam")
                nc.sync.dma_start(out=gam_sb[:], in_=gam.ap().partition_broadcast(128))
                bet_sb = consts.tile([128, 16], F16, tag="bet")
                nc.sync.dma_start(out=bet_sb[:], in_=bet.ap().partition_broadcast(128))

            def hview(ap_flat, o, rows):
                """[P, rows, 32] window at flat offset o, padded row stride 34."""
                return ap_flat[:, o: o + rows * 34].rearrange(
                    "c (h w) -> c h w", w=34
                )[:, :, 0:32]

            u_tiles = {}
            usq_tiles = {}
            T_tiles = {}
            stat_tiles = {}

            def conv(bl):
                for i in range(8):
                    m = bl * 8 + i
                    xi = imgs.tile([128, 1164], F16, tag=f"x{i}", name=f"x{i}")
                    nc.sync.dma_start(out=xi[:], in_=capsd.ap()[m])
                    u = us.tile([128, 1024], F16, tag=f"u{i}", name=f"u{i}")
                    u_tiles[m] = u
                    for chh in range(2):
                        p = ps.tile([128, 512], F32, tag="ps", name="ps")
                        base = chh * 16 * 34
                        for kx in range(3):
                            nc.tensor.matmul(
                                p[:],
                                lhsT=w_sb[:, kx * 128:(kx + 1) * 128],
                                rhs=hview(xi, base + kx, 16),
                                start=(kx == 0), stop=False)
                        for kx in range(3):
                            last = (kx == 2) and not apply_bias
                            nc.tensor.matmul(
                                p[:],
                                lhsT=w_sb[0:64, (3 + kx) * 128:(4 + kx) * 128],
                                rhs=hview(xi[0:64], base + 68 + kx, 16),
                                start=False, stop=last)
                        if apply_bias:
                            nc.tensor.matmul(
                                p[:], lhsT=cb_sb[:], rhs=ones512[:],
                                start=False, stop=True)
                        nc.scalar.activation(
                            u[:, chh * 512:(chh + 1) * 512], p[:], AF.Copy)
                    usq = us.tile([128, 1024], F16, tag=f"usq{i}", name=f"usq{i}")
                    usq_tiles[m] = usq
                    nc.gpsimd.tensor_tensor(out=usq[:], in0=u[:], in1=u[:],
                                            op=OP.mult)

            def transS(bl):
                """Per-capsule LN stats (mean, mean-square over d) on the PE
                via Mmu matmuls against u and u^2."""
                stats = Tp.tile([128, 1024], F16, tag=f"st{bl}", name=f"st{bl}")
                stat_tiles[bl] = stats
                for h in range(2):
                    sp_ = sps.tile([128, 512], F32, tag="sps", name="sps")
                    for kk in range(4):
                        k = h * 4 + kk
                        for j in range(8):
                            m = bl * 8 + j
                            co = kk * 128 + j * 16
                            nc.tensor.matmul(
                                sp_[:, co: co + 8],
                                lhsT=u_tiles[m][:, k * 128:(k + 1) * 128],
                                rhs=mmu_sb[:, 0:8], start=True, stop=True)
                            nc.tensor.matmul(
                                sp_[:, co + 8: co + 16],
                                lhsT=usq_tiles[m][:, k * 128:(k + 1) * 128],
                                rhs=mmu_sb[:, 8:16], start=True, stop=True)
                    nc.scalar.activation(
                        stats[:, h * 512:(h + 1) * 512], sp_[:], AF.Copy)

            def transT(bl):
                """Transpose u to the routing layout T[pos, (i,n,d,g)]."""
                T = Tp.tile([128, 8192], F16, tag=f"T{bl}", name=f"T{bl}")
                T_tiles[bl] = T
                for k in range(8):
                    tp_ = tps.tile([128, 1024], F16, tag="tps", name="tps")
                    for j in range(8):
                        m = bl * 8 + j
                        nc.tensor.transpose(
                            out=tp_[:, j * 128:(j + 1) * 128],
                            in_=u_tiles[m][:, k * 128:(k + 1) * 128],
                            identity=id_sb[:])
                    nc.scalar.activation(
                        T[:, k * 1024:(k + 1) * 1024], tp_[:], AF.Copy)

            # ---- merged-pair routing: both nout units of a batch as one
            # set of double-width ops; layouts keep <=4 free AP dims by
            # merging (k i) or (n d g) where needed ----

            def kig2(t):  # [128,512] cols (ki, n, g)
                return t.rearrange("p (ki n g) -> p ki n g", ki=64, n=2)

            def bc_kig2(t):  # [128,512] (ki,n,g) -> [p,ki,n,d,g]
                return (kig2(t).unsqueeze(3)
                        .broadcast_to((128, 64, 2, 16, 4)))

            def TM5(t):  # [128,8192] cols (k,i,n,d,g) -> [p,ki,n,d,g]
                return t.rearrange("p (ki n d g) -> p ki n d g", ki=64, n=2, d=16)

            def kin3(t):  # [128,8192] -> [p,k,i,(ndg)]
                return t.rearrange("p (k i ndg) -> p k i ndg", k=8, i=8)

            def itree2(src3, out_kndg):
                """Sum over i of [p,k,i,(ndg)] via tree-adds (both units)."""
                t1 = trees.tile([128, 4096], F16, tag="t4096", name="t4096")
                t13 = t1.rearrange("p (k i ndg) -> p k i ndg", k=8, i=4)
                nc.vector.tensor_tensor(out=t13, in0=src3[:, :, 0:4, :],
                                        in1=src3[:, :, 4:8, :], op=OP.add)
                t2 = trees.tile([128, 2048], F16, tag="t2048", name="t2048")
                t23 = t2.rearrange("p (k i ndg) -> p k i ndg", k=8, i=2)
                nc.vector.tensor_tensor(out=t23, in0=t13[:, :, 0:2, :],
                                        in1=t13[:, :, 2:4, :], op=OP.add)
                o3 = out_kndg.rearrange("p (k ndg) -> p k ndg", k=8).unsqueeze(2)
                nc.vector.tensor_tensor(out=o3, in0=t23[:, :, 0:1, :],
                                        in1=t23[:, :, 1:2, :], op=OP.add)

            def dtree2(src5, out_king):
                """Sum over d of [p,ki,n,d,g] via tree-adds (both units)."""
                cur = src5
                nd = 16
                szs = {8: "t4096", 4: "t2048", 2: "t1024"}
                while nd > 2:
                    nd //= 2
                    t = trees.tile([128, 64 * 2 * nd * 4], F16, tag=szs[nd],
                                   name=szs[nd])
                    t5 = t.rearrange("p (ki n d g) -> p ki n d g", ki=64, n=2,
                                     d=nd)
                    nc.vector.tensor_tensor(
                        out=t5, in0=cur[:, :, :, 0:nd, :],
                        in1=cur[:, :, :, nd:2 * nd, :], op=OP.add)
                    cur = t5
                o5 = kig2(out_king).unsqueeze(3)
                nc.vector.tensor_tensor(
                    out=o5, in0=cur[:, :, :, 0:1, :], in1=cur[:, :, :, 1:2, :],
                    op=OP.add)

            def split_tt(out5, in05, in15, op, cut):
                """One big [p,ki,n,d,g] TT, split: ki<cut on DVE, rest on Pool
                (concurrent engines, no serial coupling)."""
                if cut >= 64:
                    nc.vector.tensor_tensor(out=out5, in0=in05, in1=in15, op=op)
                    return
                nc.vector.tensor_tensor(
                    out=out5[:, 0:cut], in0=in05[:, 0:cut], in1=in15[:, 0:cut],
                    op=op)
                nc.gpsimd.tensor_tensor(
                    out=out5[:, cut:64], in0=in05[:, cut:64],
                    in1=in15[:, cut:64], op=op)

            def route_merged(bl, cut_a, cut_b):
                T = T_tiles[bl]
                T5 = TM5(T)
                stM = stat_tiles[bl].rearrange(
                    "p (ki t n g) -> p ki t n g", ki=64, t=2, n=2)
                m1v = stM[:, :, 0]   # [p,ki,n,g] mean over d
                e2v = stM[:, :, 1]   # mean of squares over d

                # LayerNorm: var = E[t^2] - m1^2 ; rstd ; n2 = m1*rstd
                mm_ = sm.tile([128, 512], F16, tag="mm_", name="mm_")
                nc.vector.tensor_tensor(out=kig2(mm_), in0=m1v, in1=m1v,
                                        op=OP.mult)
                var = sm.tile([128, 512], F16, tag="var", name="var")
                nc.vector.tensor_tensor(out=kig2(var), in0=e2v, in1=kig2(mm_),
                                        op=OP.subtract)
                rstd = sm.tile([128, 512], F16, tag="rstd", name="rstd")
                nc.scalar.activation(rstd[:], var[:], AF.Sqrt, bias=eps5[:])
                nc.vector.reciprocal(rstd[:], rstd[:])
                n2 = sm.tile([128, 512], F16, tag="n2", name="n2")
                nc.vector.tensor_tensor(out=kig2(n2), in0=m1v, in1=kig2(rstd),
                                        op=OP.mult)

                # up = (T - m1) * rstd (optionally * gamma + beta)
                q = qp.tile([128, 8192], F16, tag="q", name="q")
                split_tt(TM5(q), T5, bc_kig2(rstd), OP.mult, cut_a)
                up = qp.tile([128, 8192], F16, tag="up", name="up")
                split_tt(TM5(up), TM5(q), bc_kig2(n2), OP.subtract, cut_a)
                if apply_gb:
                    up5 = TM5(up)
                    gb = (gam_sb[:].unsqueeze(1).unsqueeze(2).unsqueeze(4)
                          .broadcast_to((128, 64, 2, 16, 4)))
                    bb_ = (bet_sb[:].unsqueeze(1).unsqueeze(2).unsqueeze(4)
                           .broadcast_to((128, 64, 2, 16, 4)))
                    nc.vector.tensor_tensor(out=up5, in0=up5, in1=gb, op=OP.mult)
                    nc.vector.tensor_tensor(out=up5, in0=up5, in1=bb_, op=OP.add)

                # S = sum_i up ; dot_i = up_i . S
                S = sm.tile([128, 1024], F16, tag="S", name="S")
                itree2(kin3(up), S)
                P = scr.tile([128, 8192], F16, tag="P", name="P")
                S_bc = (S.rearrange("p (k ndg) -> p k ndg", k=8).unsqueeze(2)
                        .broadcast_to((128, 8, 8, 128)))
                kc = cut_b // 8
                if kc >= 8:
                    nc.vector.tensor_tensor(out=kin3(P), in0=kin3(up),
                                            in1=S_bc, op=OP.mult)
                else:
                    nc.vector.tensor_tensor(
                        out=kin3(P)[:, 0:kc], in0=kin3(up)[:, 0:kc],
                        in1=S_bc[:, 0:kc], op=OP.mult)
                    nc.gpsimd.tensor_tensor(
                        out=kin3(P)[:, kc:8], in0=kin3(up)[:, kc:8],
                        in1=S_bc[:, kc:8], op=OP.mult)
                dot = sm.tile([128, 512], F16, tag="dot", name="dot")
                dtree2(TM5(P), dot)

                # rr_i = dot_i / max(||up_i||^2, 1e-8)
                rr = sm.tile([128, 512], F16, tag="rr", name="rr")
                if not apply_gb:
                    # 1/||up||^2 = (var+eps)/(16 var) = 1/16 + (eps/16)/var
                    ns_ = sm.tile([128, 512], F16, tag="ns_", name="ns_")
                    nc.vector.reciprocal(ns_[:], var[:])
                    nc.vector.tensor_scalar(
                        out=ns_[:], in0=ns_[:], scalar1=1e-5 / 16.0,
                        scalar2=1.0 / 16.0, op0=OP.mult, op1=OP.add)
                    nc.vector.tensor_tensor(out=rr[:], in0=dot[:], in1=ns_[:],
                                            op=OP.mult)
                else:
                    usq = scr.tile([128, 8192], F16, tag="P", name="usq")
                    nc.vector.tensor_tensor(out=usq[:], in0=up[:], in1=up[:],
                                            op=OP.mult)
                    nq = sm.tile([128, 512], F16, tag="nq", name="nq")
                    dtree2(TM5(usq), nq)
                    nc.vector.tensor_scalar_max(out=nq[:], in0=nq[:],
                                                scalar1=1e-8)
                    nc.vector.reciprocal(nq[:], nq[:])
                    nc.vector.tensor_tensor(out=rr[:], in0=dot[:], in1=nq[:],
                                            op=OP.mult)

                # softmax over i (shift-free: |rr| <= 8)
                es = sm.tile([128, 512], F16, tag="es", name="es")
                nc.scalar.activation(es[:], rr[:], AF.Exp)
                es4 = es.rearrange("p (k i n g) -> p k i n g", k=8, i=8, n=2)
                zt1 = trees.tile([128, 256], F16, tag="z4", name="z4")
                z14 = zt1.rearrange("p (k i n g) -> p k i n g", k=8, i=4, n=2)
                nc.vector.tensor_tensor(out=z14, in0=es4[:, :, 0:4], in1=es4[:, :, 4:8],
                                        op=OP.add)
                zt2 = trees.tile([128, 128], F16, tag="z2", name="z2")
                z24 = zt2.rearrange("p (k i n g) -> p k i n g", k=8, i=2, n=2)
                nc.vector.tensor_tensor(out=z24, in0=z14[:, :, 0:2], in1=z14[:, :, 2:4],
                                        op=OP.add)
                Z = sm.tile([128, 64], F16, tag="Z", name="Z")
                Z4 = Z.rearrange("p (k n g) -> p k n g", k=8, n=2).unsqueeze(2)
                nc.vector.tensor_tensor(out=Z4, in0=z24[:, :, 0:1], in1=z24[:, :, 1:2],
                                        op=OP.add)
                nc.vector.reciprocal(Z[:], Z[:])
                sc = sm.tile([128, 512], F16, tag="sc", name="sc")
                Zb = (Z.rearrange("p (k n g) -> p k n g", k=8, n=2).unsqueeze(2)
                      .broadcast_to((128, 8, 8, 2, 4)))
                sc4 = sc.rearrange("p (k i n g) -> p k i n g", k=8, i=8, n=2)
                nc.vector.tensor_tensor(out=sc4, in0=es4, in1=Zb, op=OP.mult)

                # s = sum_i score_i up_i ; squash over g
                P2 = scr.tile([128, 8192], F16, tag="P", name="P2")
                split_tt(TM5(P2), TM5(up), bc_kig2(sc), OP.mult, cut_b)
                s = sm.tile([128, 1024], F16, tag="s", name="s")
                itree2(kin3(P2), s)
                ssq = sm.tile([128, 1024], F16, tag="ssq", name="ssq")
                nc.vector.tensor_tensor(out=ssq[:], in0=s[:], in1=s[:], op=OP.mult)
                s4 = ssq.rearrange("p (knd g) -> p knd g", knd=256)
                gt = trees.tile([128, 512], F16, tag="g2", name="g2")
                gt4 = gt.rearrange("p (knd g) -> p knd g", knd=256)
                nc.vector.tensor_tensor(out=gt4, in0=s4[:, :, 0:2],
                                        in1=s4[:, :, 2:4], op=OP.add)
                nsq = sm.tile([128, 256], F16, tag="nsq", name="nsq")
                nsq4 = nsq[:].unsqueeze(2)
                nc.vector.tensor_tensor(out=nsq4, in0=gt4[:, :, 0:1],
                                        in1=gt4[:, :, 1:2], op=OP.add)
                sr = sm.tile([128, 256], F16, tag="sr", name="sr")
                nc.scalar.activation(sr[:], nsq[:], AF.Sqrt, bias=eps16[:])
                d1 = sm.tile([128, 256], F16, tag="d1", name="d1")
                nc.vector.scalar_tensor_tensor(
                    out=d1[:], in0=nsq[:], scalar=1.0, in1=sr[:],
                    op0=OP.add, op1=OP.mult)
                nc.vector.reciprocal(d1[:], d1[:])
                f = sm.tile([128, 256], F16, tag="f", name="f")
                nc.vector.tensor_tensor(out=f[:], in0=nsq[:], in1=d1[:], op=OP.mult)
                v = vp.tile([128, 1024], F16, tag="v", name="v")
                v4 = v.rearrange("p (knd g) -> p knd g", knd=256)
                fb = f[:].unsqueeze(2).broadcast_to((128, 256, 4))
                s44 = s.rearrange("p (knd g) -> p knd g", knd=256)
                nc.vector.tensor_tensor(out=v4, in0=s44, in1=fb, op=OP.mult)
                nc.sync.dma_start(out=outd.ap()[:, bl * 1024:(bl + 1) * 1024],
                                  in_=v[:])

            conv(0)
            transS(0)
            transT(0)
            conv(1)
            transS(1)
            transT(1)
            route_merged(0, cut_a=64, cut_b=56)
            route_merged(1, cut_a=48, cut_b=48)

    _split_sync_waits(nc)
    return nc


def _pack_weights(conv_w):
    w = np.asarray(conv_w, np.float32)
    wt = np.stack(
        [np.roll(np.rot90(w, k=r, axes=(3, 4)), r, axis=2) for r in range(4)], axis=1
    )
    W512 = np.ascontiguousarray(wt.reshape(512, 64, 3, 3), dtype=np.float32)
    packs = []
    for pi in range(4):
        Wc = W512[128 * pi: 128 * pi + 128]  # 2 nouts' channels (n,d,g)
        w_pack = np.zeros((128, 6, 128), np.float32)
        for kx in range(3):
            w_pack[0:64, kx] = Wc[:, :, 0, kx].T
            w_pack[64:128, kx] = Wc[:, :, 1, kx].T
            w_pack[0:64, 3 + kx] = Wc[:, :, 2, kx].T
        packs.append(np.ascontiguousarray(
            w_pack.reshape(128, 768), dtype=np.float16))
    return packs


def _pack_caps(capsules):
    """[4,8,16,4,32,32] f32 -> [32,128,1164] f16 (padded image + row-shifted
    copy per [128]-partition tile)."""
    x = np.asarray(capsules, np.float32).reshape(32, 64, 32, 32)
    pad = np.zeros((32, 64, 34, 34), np.float16)
    pad[:, :, 1:33, 1:33] = x.astype(np.float16)
    A = pad.reshape(32, 64, 1156)
    buf = np.zeros((32, 128, 1164), np.float16)
    buf[:, 0:64, 0:1156] = A
    buf[:, 64:128, 0:1122] = A[:, :, 34:1156]
    return buf


_CACHE = {}


def kernel(capsules, conv_w, conv_b, ln_gamma, ln_beta):
    conv_b = np.asarray(conv_b, np.float32)
    ln_gamma = np.asarray(ln_gamma, np.float32)
    ln_beta = np.asarray(ln_beta, np.float32)
    apply_bias = bool(np.any(conv_b))
    apply_gb = bool(np.any(ln_gamma != 1.0) or np.any(ln_beta != 0.0))

    key = (apply_bias, apply_gb)
    if key not in _CACHE:
        _CACHE[key] = build_program(apply_bias=apply_bias, apply_gb=apply_gb)
    nc = _CACHE[key]

    capsd = _pack_caps(capsules)
    packs = _pack_weights(conv_w)
    identity = np.eye(128, dtype=np.float16)
    mmu = np.zeros((128, 16), np.float16)
    for ch in range(128):
        nn_, gg = ch // 64, ch % 4
        for t in range(2):
            mmu[ch, t * 8 + nn_ * 4 + gg] = 1.0 / 16.0
    in_maps = []
    for c in range(8):
        beta_ = c // 4   # batch-pair
        pi = c % 4       # nout-pair
        m = {"capsd": np.ascontiguousarray(capsd[16 * beta_: 16 * beta_ + 16]),
             "w": packs[pi], "ident": identity, "mmu": mmu}
        if apply_bias:
            # channel order (n,d,g): n*64 + d*4 + g
            b_loc = np.repeat(conv_b[32 * pi: 32 * pi + 32], 4).astype(np.float16)
            m["cb"] = np.ascontiguousarray(b_loc.reshape(1, 128))
        if apply_gb:
            m["gam"] = np.ascontiguousarray(ln_gamma.reshape(1, 16), dtype=np.float16)
            m["bet"] = np.ascontiguousarray(ln_beta.reshape(1, 16), dtype=np.float16)
        in_maps.append(m)

    res = run_bass_kernel_spmd(nc, in_maps, core_ids=list(range(8)), trace=False)
    # per-core out: [128, 2048] f16 = (p, bl, n, k, d, g); position = k*128+p
    out = np.zeros((4, 8, 16, 4, 32, 32), np.float32)
    for c in range(8):
        beta_, pi = c // 4, c % 4
        r = np.asarray(res.results[c]["outd"], np.float32).reshape(128, 2, 8, 2, 16, 4)
        for bl in range(2):
            for n in range(2):
                out[2 * beta_ + bl, 2 * pi + n] = (
                    r[:, bl, :, n].transpose(2, 3, 1, 0).reshape(16, 4, 32, 32))
    return out


# revision 36
# speedup vs baseline: 1.0031x; 1.0031x over previous
"""Trainium2 Bass kernel for nn_ConvolutionalCapsules.

Sharding: core c (of 8) owns output-capsules {2*(c%4), 2*(c%4)+1} for batches
{2*(c//4), 2*(c//4)+1}. Each core runs the p4 group conv with 128 output
channels (2 nout x 16 dout x 4 rot) over its 16 images (2 batches x 8 input
capsules), then LayerNorm + degree-score routing + squash per (batch, nout).

Conv: 3x3 conv as shifted matmuls from a zero-padded fp16 SBUF image (34x34
rows, dual-copy: partitions 0-63 = padded image, 64-127 = same shifted one row
so one K=128 matmul covers two filter taps). Weights are the stationary
operand ([K, 128 out-channels]), so each of the 6 matmuls per 512 positions
retires 2x the baseline's work. PE then transposes u ([128 chan, pos]) to the
routing layout ([128 pos, chan]) in fp16 (4x cheaper than f32 transposes).

Routing runs fully in fp16 on the DVE (tensor_tensor at the 2x perf mode,
tensor_scalar at 4x); segmented reductions over d/i/g are binary tree-adds of
strided views, which beat TensorReduce ~4x. LayerNorm is algebraically folded:
up = (T - mu)*rstd, rr_i = (up_i . S) * (var+eps)/(16 var), softmax over i
(shift-free: |rr| <= 8), s = sum_i score_i up_i, squash over g.

Host packs inputs (pad + dual-copy + fp16) and unpacks the [pos, (b,n,k,d,g)]
fp16 output, so every DMA moves contiguous >=512B lines.
"""

import numpy as np
from contextlib import ExitStack

import concourse.bass as bass
import concourse.tile as tile
from concourse import mybir
from concourse.bass_utils import run_bass_kernel_spmd

F16 = mybir.dt.float16
F32 = mybir.dt.float32
AF = mybir.ActivationFunctionType
OP = mybir.AluOpType

_ENGINES = {
    mybir.EngineType.PE,
    mybir.EngineType.Activation,
    mybir.EngineType.Pool,
    mybir.EngineType.DVE,
    mybir.EngineType.SP,
}


def _split_sync_waits(nc):
    """This walrus build accepts a single embedded sync-wait per instruction;
    hoist extras onto preceding NoOps on the same engine (ge-imm waits commute)."""
    for f in nc.m.functions:
        for bb in f.blocks:
            newl = []
            changed = False
            for inst in list(bb.instructions):
                si = inst.sync_info
                waits = list(si.on_wait) if si and si.on_wait else []
                if len(waits) > 1 and inst.engine in _ENGINES:
                    changed = True
                    for k, w in enumerate(waits[:-1]):
                        newl.append(
                            mybir.InstNoOp(
                                name=f"{inst.name}-ws{k}",
                                ins=[],
                                outs=[],
                                engine=inst.engine,
                                sync_info=mybir.SyncInfo(on_wait=[w], on_update=[]),
                            )
                        )
                    si.on_wait = waits[-1:]
                    inst.sync_info = si
                newl.append(inst)
            if changed:
                bb.instructions = newl


def build_program(apply_bias=False, apply_gb=False):
    nc = bass.Bass(trn_type="TRN2")
    capsd = nc.dram_tensor("capsd", [16, 128, 1164], F16, kind="ExternalInput")
    w = nc.dram_tensor("w", [128, 768], F16, kind="ExternalInput")
    ident = nc.dram_tensor("ident", [128, 128], F16, kind="ExternalInput")
    mmu = nc.dram_tensor("mmu", [128, 16], F16, kind="ExternalInput")
    if apply_bias:
        cb = nc.dram_tensor("cb", [1, 128], F16, kind="ExternalInput")
    if apply_gb:
        gam = nc.dram_tensor("gam", [1, 16], F16, kind="ExternalInput")
        bet = nc.dram_tensor("bet", [1, 16], F16, kind="ExternalInput")
    outd = nc.dram_tensor("outd", [128, 2048], F16, kind="ExternalOutput")

    with tile.TileContext(nc) as tc:
        with nc.allow_low_precision(reason="fp16 routing; 2e-2 rel tolerance"), \
             ExitStack() as ctx:
            consts = ctx.enter_context(tc.tile_pool(name="consts", bufs=1))
            imgs = ctx.enter_context(tc.tile_pool(name="imgs", bufs=1))
            us = ctx.enter_context(tc.tile_pool(name="us", bufs=1))
            ps = ctx.enter_context(tc.tile_pool(name="ps", bufs=4, space="PSUM"))
            tps = ctx.enter_context(tc.tile_pool(name="tps", bufs=3, space="PSUM"))
            sps = ctx.enter_context(tc.tile_pool(name="sps", bufs=1, space="PSUM"))
            Tp = ctx.enter_context(tc.tile_pool(name="Tp", bufs=1))
            qp = ctx.enter_context(tc.tile_pool(name="qp", bufs=1))
            scr = ctx.enter_context(tc.tile_pool(name="scr", bufs=2))
            trees = ctx.enter_context(tc.tile_pool(name="trees", bufs=1))
            sm = ctx.enter_context(tc.tile_pool(name="sm", bufs=2))
            vp = ctx.enter_context(tc.tile_pool(name="vp", bufs=2))

            w_sb = consts.tile([128, 768], F16, tag="w")
            nc.sync.dma_start(out=w_sb[:], in_=w.ap())
            id_sb = consts.tile([128, 128], F16, tag="ident")
            nc.sync.dma_start(out=id_sb[:], in_=ident.ap())
            mmu_sb = consts.tile([128, 16], F16, tag="mmu")
            nc.sync.dma_start(out=mmu_sb[:], in_=mmu.ap())
            eps5 = consts.tile([128, 1], F32, tag="eps5")
            nc.vector.memset(eps5[:], 1e-5)
            eps16 = consts.tile([128, 1], F32, tag="eps16")
            nc.vector.memset(eps16[:], 1e-16)
            if apply_bias:
                cb_sb = consts.tile([1, 128], F16, tag="cb")
                nc.sync.dma_start(out=cb_sb[:], in_=cb.ap())
                ones512 = consts.tile([1, 512], F16, tag="ones512")
                nc.vector.memset(ones512[:], 1.0)
            if apply_gb:
                gam_sb = consts.tile([128, 16], F16, tag="gam")
                nc.sync.dma_start(out=gam_sb[:], in_=gam.ap().partition_broadcast(128))
                bet_sb = consts.tile([128, 16], F16, tag="bet")
                nc.sync.dma_start(out=bet_sb[:], in_=bet.ap().partition_broadcast(128))

            def hview(ap_flat, o, rows):
                """[P, rows, 32] window at flat offset o, padded row stride 34."""
                return ap_flat[:, o: o + rows * 34].rearrange(
                    "c (h w) -> c h w", w=34
                )[:, :, 0:32]

            u_tiles = {}
            usq_tiles = {}
            T_tiles = {}
            stat_tiles = {}

            def conv(bl):
                for i in range(8):
                    m = bl * 8 + i
                    xi = imgs.tile([128, 1164], F16, tag=f"x{i}", name=f"x{i}")
                    nc.sync.dma_start(out=xi[:], in_=capsd.ap()[m])
                    u = us.tile([128, 1024], F16, tag=f"u{i}", name=f"u{i}")
                    u_tiles[m] = u
                    for chh in range(2):
                        p = ps.tile([128, 512], F32, tag="ps", name="ps")
                        base = chh * 16 * 34
                        for kx in range(3):
                            nc.tensor.matmul(
                                p[:],
                                lhsT=w_sb[:, kx * 128:(kx + 1) * 128],
                                rhs=hview(xi, base + kx, 16),
                                start=(kx == 0), stop=False)
                        for kx in range(3):
                            last = (kx == 2) and not apply_bias
                            nc.tensor.matmul(
                                p[:],
                                lhsT=w_sb[0:64, (3 + kx) * 128:(4 + kx) * 128],
                                rhs=hview(xi[0:64], base + 68 + kx, 16),
                                start=False, stop=last)
                        if apply_bias:
                            nc.tensor.matmul(
                                p[:], lhsT=cb_sb[:], rhs=ones512[:],
                                start=False, stop=True)
                        nc.scalar.activation(
                            u[:, chh * 512:(chh + 1) * 512], p[:], AF.Copy)
                    usq = us.tile([128, 1024], F16, tag=f"usq{i}", name=f"usq{i}")
                    usq_tiles[m] = usq
                    nc.gpsimd.tensor_tensor(out=usq[:], in0=u[:], in1=u[:],
                                            op=OP.mult)

            def transS(bl):
                """Per-capsule LN stats (mean, mean-square over d) on the PE
                via Mmu matmuls against u and u^2."""
                stats = Tp.tile([128, 1024], F16, tag=f"st{bl}", name=f"st{bl}")
                stat_tiles[bl] = stats
                for h in range(2):
                    sp_ = sps.tile([128, 512], F32, tag="sps", name="sps")
                    for kk in range(4):
                        k = h * 4 + kk
                        for j in range(8):
                            m = bl * 8 + j
                            co = kk * 128 + j * 16
                            nc.tensor.matmul(
                                sp_[:, co: co + 8],
                                lhsT=u_tiles[m][:, k * 128:(k + 1) * 128],
                                rhs=mmu_sb[:, 0:8], start=True, stop=True)
                            nc.tensor.matmul(
                                sp_[:, co + 8: co + 16],
                                lhsT=usq_tiles[m][:, k * 128:(k + 1) * 128],
                                rhs=mmu_sb[:, 8:16], start=True, stop=True)
                    nc.scalar.activation(
                        stats[:, h * 512:(h + 1) * 512], sp_[:], AF.Copy)

            def transT(bl):
                """Transpose u to the routing layout T[pos, (i,n,d,g)]."""
                T = Tp.tile([128, 8192], F16, tag=f"T{bl}", name=f"T{bl}")
                T_tiles[bl] = T
                for k in range(8):
                    tp_ = tps.tile([128, 1024], F16, tag="tps", name="tps")
                    for j in range(8):
                        m = bl * 8 + j
                        nc.tensor.transpose(
                            out=tp_[:, j * 128:(j + 1) * 128],
                            in_=u_tiles[m][:, k * 128:(k + 1) * 128],
                            identity=id_sb[:])
                    nc.scalar.activation(
                        T[:, k * 1024:(k + 1) * 1024], tp_[:], AF.Copy)

            # ---- merged-pair routing: both nout units of a batch as one
            # set of double-width ops; layouts keep <=4 free AP dims by
            # merging (k i) or (n d g) where needed ----

            def kig2(t):  # [128,512] cols (ki, n, g)
                return t.rearrange("p (ki n g) -> p ki n g", ki=64, n=2)

            def bc_kig2(t):  # [128,512] (ki,n,g) -> [p,ki,n,d,g]
                return (kig2(t).unsqueeze(3)
                        .broadcast_to((128, 64, 2, 16, 4)))

            def TM5(t):  # [128,8192] cols (k,i,n,d,g) -> [p,ki,n,d,g]
                return t.rearrange("p (ki n d g) -> p ki n d g", ki=64, n=2, d=16)

            def kin3(t):  # [128,8192] -> [p,k,i,(ndg)]
                return t.rearrange("p (k i ndg) -> p k i ndg", k=8, i=8)

            def itree2(src3, out_kndg):
                """Sum over i of [p,k,i,(ndg)] via tree-adds (both units)."""
                t1 = trees.tile([128, 4096], F16, tag="t4096", name="t4096")
                t13 = t1.rearrange("p (k i ndg) -> p k i ndg", k=8, i=4)
                nc.vector.tensor_tensor(out=t13, in0=src3[:, :, 0:4, :],
                                        in1=src3[:, :, 4:8, :], op=OP.add)
                t2 = trees.tile([128, 2048], F16, tag="t2048", name="t2048")
                t23 = t2.rearrange("p (k i ndg) -> p k i ndg", k=8, i=2)
                nc.vector.tensor_tensor(out=t23, in0=t13[:, :, 0:2, :],
                                        in1=t13[:, :, 2:4, :], op=OP.add)
                o3 = out_kndg.rearrange("p (k ndg) -> p k ndg", k=8).unsqueeze(2)
                nc.vector.tensor_tensor(out=o3, in0=t23[:, :, 0:1, :],
                                        in1=t23[:, :, 1:2, :], op=OP.add)

            def dtree2(src5, out_king):
                """Sum over d of [p,ki,n,d,g] via tree-adds (both units)."""
                cur = src5
                nd = 16
                szs = {8: "t4096", 4: "t2048", 2: "t1024"}
                while nd > 2:
                    nd //= 2
                    t = trees.tile([128, 64 * 2 * nd * 4], F16, tag=szs[nd],
                                   name=szs[nd])
                    t5 = t.rearrange("p (ki n d g) -> p ki n d g", ki=64, n=2,
                                     d=nd)
                    nc.vector.tensor_tensor(
                        out=t5, in0=cur[:, :, :, 0:nd, :],
                        in1=cur[:, :, :, nd:2 * nd, :], op=OP.add)
                    cur = t5
                o5 = kig2(out_king).unsqueeze(3)
                nc.vector.tensor_tensor(
                    out=o5, in0=cur[:, :, :, 0:1, :], in1=cur[:, :, :, 1:2, :],
                    op=OP.add)

            def split_tt(out5, in05, in15, op, cut):
                """One big [p,ki,n,d,g] TT, split: ki<cut on DVE, rest on Pool
                (concurrent engines, no serial coupling)."""
                if cut >= 64:
                    nc.vector.tensor_tensor(out=out5, in0=in05, in1=in15, op=op)
                    return
                nc.vector.tensor_tensor(
                    out=out5[:, 0:cut], in0=in05[:, 0:cut], in1=in15[:, 0:cut],
                    op=op)
                nc.gpsimd.tensor_tensor(
                    out=out5[:, cut:64], in0=in05[:, cut:64],
                    in1=in15[:, cut:64], op=op)

            def route_merged(bl, cut_a, cut_b):
                T = T_tiles[bl]
                T5 = TM5(T)
                stM = stat_tiles[bl].rearrange(
                    "p (ki t n g) -> p ki t n g", ki=64, t=2, n=2)
                m1v = stM[:, :, 0]   # [p,ki,n,g] mean over d
                e2v = stM[:, :, 1]   # mean of squares over d

                # LayerNorm: var = E[t^2] - m1^2 ; rstd ; n2 = m1*rstd
                mm_ = sm.tile([128, 512], F16, tag="mm_", name="mm_")
                nc.vector.tensor_tensor(out=kig2(mm_), in0=m1v, in1=m1v,
                                        op=OP.mult)
                var = sm.tile([128, 512], F16, tag="var", name="var")
                nc.vector.tensor_tensor(out=kig2(var), in0=e2v, in1=kig2(mm_),
                                        op=OP.subtract)
                rstd = sm.tile([128, 512], F16, tag="rstd", name="rstd")
                nc.scalar.activation(rstd[:], var[:], AF.Sqrt, bias=eps5[:])
                nc.vector.reciprocal(rstd[:], rstd[:])
                n2 = sm.tile([128, 512], F16, tag="n2", name="n2")
                nc.vector.tensor_tensor(out=kig2(n2), in0=m1v, in1=kig2(rstd),
                                        op=OP.mult)

                # up = (T - m1) * rstd (optionally * gamma + beta)
                q = qp.tile([128, 8192], F16, tag="q", name="q")
                split_tt(TM5(q), T5, bc_kig2(rstd), OP.mult, cut_a)
                up = qp.tile([128, 8192], F16, tag="up", name="up")
                split_tt(TM5(up), TM5(q), bc_kig2(n2), OP.subtract, cut_a)
                if apply_gb:
                    up5 = TM5(up)
                    gb = (gam_sb[:].unsqueeze(1).unsqueeze(2).unsqueeze(4)
                          .broadcast_to((128, 64, 2, 16, 4)))
                    bb_ = (bet_sb[:].unsqueeze(1).unsqueeze(2).unsqueeze(4)
                           .broadcast_to((128, 64, 2, 16, 4)))
                    nc.vector.tensor_tensor(out=up5, in0=up5, in1=gb, op=OP.mult)
                    nc.vector.tensor_tensor(out=up5, in0=up5, in1=bb_, op=OP.add)

                # S = sum_i up ; dot_i = up_i . S
                S = sm.tile([128, 1024], F16, tag="S", name="S")
                itree2(kin3(up), S)
                P = scr.tile([128, 8192], F16, tag="P", name="P")
                S_bc = (S.rearrange("p (k ndg) -> p k ndg", k=8).unsqueeze(2)
                        .broadcast_to((128, 8, 8, 128)))
                kc = cut_b // 8
                if kc >= 8:
                    nc.vector.tensor_tensor(out=kin3(P), in0=kin3(up),
                                            in1=S_bc, op=OP.mult)
                else:
                    nc.vector.tensor_tensor(
                        out=kin3(P)[:, 0:kc], in0=kin3(up)[:, 0:kc],
                        in1=S_bc[:, 0:kc], op=OP.mult)
                    nc.gpsimd.tensor_tensor(
                        out=kin3(P)[:, kc:8], in0=kin3(up)[:, kc:8],
                        in1=S_bc[:, kc:8], op=OP.mult)
                dot = sm.tile([128, 512], F16, tag="dot", name="dot")
                dtree2(TM5(P), dot)

                # rr_i = dot_i / max(||up_i||^2, 1e-8)
                rr = sm.tile([128, 512], F16, tag="rr", name="rr")
                if not apply_gb:
                    # 1/||up||^2 = (var+eps)/(16 var) = 1/16 + (eps/16)/var
                    ns_ = sm.tile([128, 512], F16, tag="ns_", name="ns_")
                    nc.vector.reciprocal(ns_[:], var[:])
                    nc.vector.tensor_scalar(
                        out=ns_[:], in0=ns_[:], scalar1=1e-5 / 16.0,
                        scalar2=1.0 / 16.0, op0=OP.mult, op1=OP.add)
                    nc.vector.tensor_tensor(out=rr[:], in0=dot[:], in1=ns_[:],
                                            op=OP.mult)
                else:
                    usq = scr.tile([128, 8192], F16, tag="P", name="usq")
                    nc.vector.tensor_tensor(out=usq[:], in0=up[:], in1=up[:],
                                            op=OP.mult)
                    nq = sm.tile([128, 512], F16, tag="nq", name="nq")
                    dtree2(TM5(usq), nq)
                    nc.vector.tensor_scalar_max(out=nq[:], in0=nq[:],
                                                scalar1=1e-8)
                    nc.vector.reciprocal(nq[:], nq[:])
                    nc.vector.tensor_tensor(out=rr[:], in0=dot[:], in1=nq[:],
                                            op=OP.mult)

                # softmax over i (shift-free: |rr| <= 8)
                es = sm.tile([128, 512], F16, tag="es", name="es")
                nc.scalar.activation(es[:], rr[:], AF.Exp)
                es4 = es.rearrange("p (k i n g) -> p k i n g", k=8, i=8, n=2)
                zt1 = trees.tile([128, 256], F16, tag="z4", name="z4")
                z14 = zt1.rearrange("p (k i n g) -> p k i n g", k=8, i=4, n=2)
                nc.vector.tensor_tensor(out=z14, in0=es4[:, :, 0:4], in1=es4[:, :, 4:8],
                                        op=OP.add)
                zt2 = trees.tile([128, 128], F16, tag="z2", name="z2")
                z24 = zt2.rearrange("p (k i n g) -> p k i n g", k=8, i=2, n=2)
                nc.vector.tensor_tensor(out=z24, in0=z14[:, :, 0:2], in1=z14[:, :, 2:4],
                                        op=OP.add)
                Z = sm.tile([128, 64], F16, tag="Z", name="Z")
                Z4 = Z.rearrange("p (k n g) -> p k n g", k=8, n=2).unsqueeze(2)
                nc.vector.tensor_tensor(out=Z4, in0=z24[:, :, 0:1], in1=z24[:, :, 1:2],
                                        op=OP.add)
                nc.vector.reciprocal(Z[:], Z[:])
                sc = sm.tile([128, 512], F16, tag="sc", name="sc")
                Zb = (Z.rearrange("p (k n g) -> p k n g", k=8, n=2).unsqueeze(2)
                      .broadcast_to((128, 8, 8, 2, 4)))
                sc4 = sc.rearrange("p (k i n g) -> p k i n g", k=8, i=8, n=2)
                nc.vector.tensor_tensor(out=sc4, in0=es4, in1=Zb, op=OP.mult)

                # s = sum_i score_i up_i ; squash over g
                P2 = scr.tile([128, 8192], F16, tag="P", name="P2")
                split_tt(TM5(P2), TM5(up), bc_kig2(sc), OP.mult, cut_b)
                s = sm.tile([128, 1024], F16, tag="s", name="s")
                itree2(kin3(P2), s)
                ssq = sm.tile([128, 1024], F16, tag="ssq", name="ssq")
                nc.vector.tensor_tensor(out=ssq[:], in0=s[:], in1=s[:], op=OP.mult)
                s4 = ssq.rearrange("p (knd g) -> p knd g", knd=256)
                gt = trees.tile([128, 512], F16, tag="g2", name="g2")
                gt4 = gt.rearrange("p (knd g) -> p knd g", knd=256)
                nc.vector.tensor_tensor(out=gt4, in0=s4[:, :, 0:2],
                                        in1=s4[:, :, 2:4], op=OP.add)
                nsq = sm.tile([128, 256], F16, tag="nsq", name="nsq")
                nsq4 = nsq[:].unsqueeze(2)
                nc.vector.tensor_tensor(out=nsq4, in0=gt4[:, :, 0:1],
                                        in1=gt4[:, :, 1:2], op=OP.add)
                sr = sm.tile([128, 256], F16, tag="sr", name="sr")
                nc.scalar.activation(sr[:], nsq[:], AF.Sqrt, bias=eps16[:])
                d1 = sm.tile([128, 256], F16, tag="d1", name="d1")
                nc.vector.scalar_tensor_tensor(
                    out=d1[:], in0=nsq[:], scalar=1.0, in1=sr[:],
                    op0=OP.add, op1=OP.mult)
                nc.vector.reciprocal(d1[:], d1[:])
                f = sm.tile([128, 256], F16, tag="f", name="f")
                nc.vector.tensor_tensor(out=f[:], in0=nsq[:], in1=d1[:], op=OP.mult)
                v = vp.tile([128, 1024], F16, tag="v", name="v")
                v4 = v.rearrange("p (knd g) -> p knd g", knd=256)
                fb = f[:].unsqueeze(2).broadcast_to((128, 256, 4))
                s44 = s.rearrange("p (knd g) -> p knd g", knd=256)
                nc.vector.tensor_tensor(out=v4, in0=s44, in1=fb, op=OP.mult)
                nc.sync.dma_start(out=outd.ap()[:, bl * 1024:(bl + 1) * 1024],
                                  in_=v[:])

            conv(0)
            transS(0)
            transT(0)
            conv(1)
            transS(1)
            transT(1)
            route_merged(0, cut_a=64, cut_b=56)
            route_merged(1, cut_a=48, cut_b=48)

    _split_sync_waits(nc)
    return nc


def _pack_weights(conv_w):
    w = np.asarray(conv_w, np.float32)
    wt = np.stack(
        [np.roll(np.rot90(w, k=r, axes=(3, 4)), r, axis=2) for r in range(4)], axis=1
    )
    W512 = np.ascontiguousarray(wt.reshape(512, 64, 3, 3), dtype=np.float32)
    packs = []
    for pi in range(4):
        Wc = W512[128 * pi: 128 * pi + 128]  # 2 nouts' channels (n,d,g)
        w_pack = np.zeros((128, 6, 128), np.float32)
        for kx in range(3):
            w_pack[0:64, kx] = Wc[:, :, 0, kx].T
            w_pack[64:128, kx] = Wc[:, :, 1, kx].T
            w_pack[0:64, 3 + kx] = Wc[:, :, 2, kx].T
        packs.append(np.ascontiguousarray(
            w_pack.reshape(128, 768), dtype=np.float16))
    return packs


def _pack_caps(capsules):
    """[4,8,16,4,32,32] f32 -> [32,128,1164] f16 (padded image + row-shifted
    copy per [128]-partition tile)."""
    x = np.asarray(capsules, np.float32).reshape(32, 64, 32, 32)
    pad = np.zeros((32, 64, 34, 34), np.float16)
    pad[:, :, 1:33, 1:33] = x.astype(np.float16)
    A = pad.reshape(32, 64, 1156)
    buf = np.zeros((32, 128, 1164), np.float16)
    buf[:, 0:64, 0:1156] = A
    buf[:, 64:128, 0:1122] = A[:, :, 34:1156]
    return buf


_CACHE = {}


def kernel(capsules, conv_w, conv_b, ln_gamma, ln_beta):
    conv_b = np.asarray(conv_b, np.float32)
    ln_gamma = np.asarray(ln_gamma, np.float32)
    ln_beta = np.asarray(ln_beta, np.float32)
    apply_bias = bool(np.any(conv_b))
    apply_gb = bool(np.any(ln_gamma != 1.0) or np.any(ln_beta != 0.0))

    key = (apply_bias, apply_gb)
    if key not in _CACHE:
        _CACHE[key] = build_program(apply_bias=apply_bias, apply_gb=apply_gb)
    nc = _CACHE[key]

    capsd = _pack_caps(capsules)
    packs = _pack_weights(conv_w)
    identity = np.eye(128, dtype=np.float16)
    mmu = np.zeros((128, 16), np.float16)
    for ch in range(128):
        nn_, gg = ch // 64, ch % 4
        for t in range(2):
            mmu[ch, t * 8 + nn_ * 4 + gg] = 1.0 / 16.0
    in_maps = []
    for c in range(8):
        beta_ = c // 4   # batch-pair
        pi = c % 4       # nout-pair
        m = {"capsd": np.ascontiguousarray(capsd[16 * beta_: 16 * beta_ + 16]),
             "w": packs[pi], "ident": identity, "mmu": mmu}
        if apply_bias:
            # channel order (n,d,g): n*64 + d*4 + g
            b_loc = np.repeat(conv_b[32 * pi: 32 * pi + 32], 4).astype(np.float16)
            m["cb"] = np.ascontiguousarray(b_loc.reshape(1, 128))
        if apply_gb:
            m["gam"] = np.ascontiguousarray(ln_gamma.reshape(1, 16), dtype=np.float16)
            m["bet"] = np.ascontiguousarray(ln_beta.reshape(1, 16), dtype=np.float16)
        in_maps.append(m)

    res = run_bass_kernel_spmd(nc, in_maps, core_ids=list(range(8)), trace=False)
    # per-core out: [128, 2048] f16 = (p, bl, n, k, d, g); position = k*128+p
    out = np.zeros((4, 8, 16, 4, 32, 32), np.float32)
    for c in range(8):
        beta_, pi = c // 4, c % 4
        r = np.asarray(res.results[c]["outd"], np.float32).reshape(128, 2, 8, 2, 16, 4)
        for bl in range(2):
            for n in range(2):
                out[2 * beta_ + bl, 2 * pi + n] = (
                    r[:, bl, :, n].transpose(2, 3, 1, 0).reshape(16, 4, 32, 32))
    return out


# revision 37
# speedup vs baseline: 1.0151x; 1.0119x over previous
"""Trainium2 Bass kernel for nn_ConvolutionalCapsules.

Sharding: core c (of 8) owns output-capsules {2*(c%4), 2*(c%4)+1} for batches
{2*(c//4), 2*(c//4)+1}. Each core runs the p4 group conv with 128 output
channels (2 nout x 16 dout x 4 rot) over its 16 images (2 batches x 8 input
capsules), then LayerNorm + degree-score routing + squash per (batch, nout).

Conv: 3x3 conv as shifted matmuls from a zero-padded fp16 SBUF image (34x34
rows, dual-copy: partitions 0-63 = padded image, 64-127 = same shifted one row
so one K=128 matmul covers two filter taps). Weights are the stationary
operand ([K, 128 out-channels]), so each of the 6 matmuls per 512 positions
retires 2x the baseline's work. PE then transposes u ([128 chan, pos]) to the
routing layout ([128 pos, chan]) in fp16 (4x cheaper than f32 transposes).

Routing runs fully in fp16 on the DVE (tensor_tensor at the 2x perf mode,
tensor_scalar at 4x); segmented reductions over d/i/g are binary tree-adds of
strided views, which beat TensorReduce ~4x. LayerNorm is algebraically folded:
up = (T - mu)*rstd, rr_i = (up_i . S) * (var+eps)/(16 var), softmax over i
(shift-free: |rr| <= 8), s = sum_i score_i up_i, squash over g.

Host packs inputs (pad + dual-copy + fp16) and unpacks the [pos, (b,n,k,d,g)]
fp16 output, so every DMA moves contiguous >=512B lines.
"""

import numpy as np
from contextlib import ExitStack

import concourse.bass as bass
import concourse.tile as tile
from concourse import mybir
from concourse.bass_utils import run_bass_kernel_spmd

F16 = mybir.dt.float16
F32 = mybir.dt.float32
AF = mybir.ActivationFunctionType
OP = mybir.AluOpType

_ENGINES = {
    mybir.EngineType.PE,
    mybir.EngineType.Activation,
    mybir.EngineType.Pool,
    mybir.EngineType.DVE,
    mybir.EngineType.SP,
}


def _split_sync_waits(nc):
    """This walrus build accepts a single embedded sync-wait per instruction;
    hoist extras onto preceding NoOps on the same engine (ge-imm waits commute)."""
    for f in nc.m.functions:
        for bb in f.blocks:
            newl = []
            changed = False
            for inst in list(bb.instructions):
                si = inst.sync_info
                waits = list(si.on_wait) if si and si.on_wait else []
                if len(waits) > 1 and inst.engine in _ENGINES:
                    changed = True
                    for k, w in enumerate(waits[:-1]):
                        newl.append(
                            mybir.InstNoOp(
                                name=f"{inst.name}-ws{k}",
                                ins=[],
                                outs=[],
                                engine=inst.engine,
                                sync_info=mybir.SyncInfo(on_wait=[w], on_update=[]),
                            )
                        )
                    si.on_wait = waits[-1:]
                    inst.sync_info = si
                newl.append(inst)
            if changed:
                bb.instructions = newl


def build_program(apply_bias=False, apply_gb=False):
    nc = bass.Bass(trn_type="TRN2")
    capsd = nc.dram_tensor("capsd", [16, 128, 1164], F16, kind="ExternalInput")
    w = nc.dram_tensor("w", [128, 768], F16, kind="ExternalInput")
    ident = nc.dram_tensor("ident", [128, 128], F16, kind="ExternalInput")
    mmu = nc.dram_tensor("mmu", [128, 16], F16, kind="ExternalInput")
    if apply_bias:
        cb = nc.dram_tensor("cb", [1, 128], F16, kind="ExternalInput")
    if apply_gb:
        gam = nc.dram_tensor("gam", [1, 16], F16, kind="ExternalInput")
        bet = nc.dram_tensor("bet", [1, 16], F16, kind="ExternalInput")
    outd = nc.dram_tensor("outd", [128, 2048], F16, kind="ExternalOutput")

    with tile.TileContext(nc) as tc:
        with nc.allow_low_precision(reason="fp16 routing; 2e-2 rel tolerance"), \
             ExitStack() as ctx:
            consts = ctx.enter_context(tc.tile_pool(name="consts", bufs=1))
            imgs = ctx.enter_context(tc.tile_pool(name="imgs", bufs=1))
            us = ctx.enter_context(tc.tile_pool(name="us", bufs=1))
            ps = ctx.enter_context(tc.tile_pool(name="ps", bufs=4, space="PSUM"))
            tps = ctx.enter_context(tc.tile_pool(name="tps", bufs=3, space="PSUM"))
            sps = ctx.enter_context(tc.tile_pool(name="sps", bufs=1, space="PSUM"))
            Tp = ctx.enter_context(tc.tile_pool(name="Tp", bufs=1))
            qp = ctx.enter_context(tc.tile_pool(name="qp", bufs=1))
            scr = ctx.enter_context(tc.tile_pool(name="scr", bufs=2))
            trees = ctx.enter_context(tc.tile_pool(name="trees", bufs=1))
            sm = ctx.enter_context(tc.tile_pool(name="sm", bufs=2))
            vp = ctx.enter_context(tc.tile_pool(name="vp", bufs=2))

            w_sb = consts.tile([128, 768], F16, tag="w")
            nc.sync.dma_start(out=w_sb[:], in_=w.ap())
            id_sb = consts.tile([128, 128], F16, tag="ident")
            nc.sync.dma_start(out=id_sb[:], in_=ident.ap())
            mmu_sb = consts.tile([128, 16], F16, tag="mmu")
            nc.sync.dma_start(out=mmu_sb[:], in_=mmu.ap())
            eps5 = consts.tile([128, 1], F32, tag="eps5")
            nc.vector.memset(eps5[:], 1e-5)
            eps16 = consts.tile([128, 1], F32, tag="eps16")
            nc.vector.memset(eps16[:], 1e-16)
            if apply_bias:
                cb_sb = consts.tile([1, 128], F16, tag="cb")
                nc.sync.dma_start(out=cb_sb[:], in_=cb.ap())
                ones512 = consts.tile([1, 512], F16, tag="ones512")
                nc.vector.memset(ones512[:], 1.0)
            if apply_gb:
                gam_sb = consts.tile([128, 16], F16, tag="gam")
                nc.sync.dma_start(out=gam_sb[:], in_=gam.ap().partition_broadcast(128))
                bet_sb = consts.tile([128, 16], F16, tag="bet")
                nc.sync.dma_start(out=bet_sb[:], in_=bet.ap().partition_broadcast(128))

            def hview(ap_flat, o, rows):
                """[P, rows, 32] window at flat offset o, padded row stride 34."""
                return ap_flat[:, o: o + rows * 34].rearrange(
                    "c (h w) -> c h w", w=34
                )[:, :, 0:32]

            u_tiles = {}
            usq_tiles = {}
            T_tiles = {}
            stat_tiles = {}

            x_tiles = {}

            def conv_pass(bl, chh):
                """One 512-position chunk (4 k-blocks) of the conv for all 8
                images of batch bl. chh=0 covers k 0-3, chh=1 covers k 4-7."""
                base = chh * 16 * 34
                for i in range(8):
                    m = bl * 8 + i
                    if chh == 0:
                        xi = imgs.tile([128, 1164], F16, tag=f"x{i}", name=f"x{i}")
                        nc.sync.dma_start(out=xi[:], in_=capsd.ap()[m])
                        x_tiles[m] = xi
                        u = us.tile([128, 1024], F16, tag=f"u{i}", name=f"u{i}")
                        u_tiles[m] = u
                        usq = us.tile([128, 1024], F16, tag=f"usq{i}",
                                      name=f"usq{i}")
                        usq_tiles[m] = usq
                    xi, u, usq = x_tiles[m], u_tiles[m], usq_tiles[m]
                    p = ps.tile([128, 512], F32, tag="ps", name="ps")
                    for kx in range(3):
                        nc.tensor.matmul(
                            p[:],
                            lhsT=w_sb[:, kx * 128:(kx + 1) * 128],
                            rhs=hview(xi, base + kx, 16),
                            start=(kx == 0), stop=False)
                    for kx in range(3):
                        last = (kx == 2) and not apply_bias
                        nc.tensor.matmul(
                            p[:],
                            lhsT=w_sb[0:64, (3 + kx) * 128:(4 + kx) * 128],
                            rhs=hview(xi[0:64], base + 68 + kx, 16),
                            start=False, stop=last)
                    if apply_bias:
                        nc.tensor.matmul(
                            p[:], lhsT=cb_sb[:], rhs=ones512[:],
                            start=False, stop=True)
                    lo = chh * 512
                    nc.scalar.activation(u[:, lo:lo + 512], p[:], AF.Copy)
                    nc.gpsimd.tensor_tensor(
                        out=usq[:, lo:lo + 512], in0=u[:, lo:lo + 512],
                        in1=u[:, lo:lo + 512], op=OP.mult)

            def transS_half(bl, h):
                """LN stats (mean, mean-square over d) for k-blocks h*4..h*4+3
                on the PE via Mmu matmuls against u and u^2."""
                if h == 0:
                    stats = Tp.tile([128, 1024], F16, tag=f"st{bl}",
                                    name=f"st{bl}")
                    stat_tiles[bl] = stats
                stats = stat_tiles[bl]
                sp_ = sps.tile([128, 512], F32, tag="sps", name="sps")
                for kk in range(4):
                    k = h * 4 + kk
                    for j in range(8):
                        m = bl * 8 + j
                        co = kk * 128 + j * 16
                        nc.tensor.matmul(
                            sp_[:, co: co + 8],
                            lhsT=u_tiles[m][:, k * 128:(k + 1) * 128],
                            rhs=mmu_sb[:, 0:8], start=True, stop=True)
                        nc.tensor.matmul(
                            sp_[:, co + 8: co + 16],
                            lhsT=usq_tiles[m][:, k * 128:(k + 1) * 128],
                            rhs=mmu_sb[:, 8:16], start=True, stop=True)
                nc.scalar.activation(
                    stats[:, h * 512:(h + 1) * 512], sp_[:], AF.Copy)

            def transT_half(bl, h):
                """Transpose k-blocks h*4..h*4+3 of u into T[pos, (i,n,d,g)]."""
                if h == 0:
                    T = Tp.tile([128, 8192], F16, tag=f"T{bl}", name=f"T{bl}")
                    T_tiles[bl] = T
                T = T_tiles[bl]
                for kk in range(4):
                    k = h * 4 + kk
                    tp_ = tps.tile([128, 1024], F16, tag="tps", name="tps")
                    for j in range(8):
                        m = bl * 8 + j
                        nc.tensor.transpose(
                            out=tp_[:, j * 128:(j + 1) * 128],
                            in_=u_tiles[m][:, k * 128:(k + 1) * 128],
                            identity=id_sb[:])
                    nc.scalar.activation(
                        T[:, k * 1024:(k + 1) * 1024], tp_[:], AF.Copy)

            # ---- half-pair routing: both nout units of a batch for 4 of the
            # 8 position-blocks, as one set of double-width ops. Four such
            # chains pipeline against each other and the conv phases. ----

            def kig2(t):  # [128,256] cols (ki32, n, g)
                return t.rearrange("p (ki n g) -> p ki n g", ki=32, n=2)

            def bc_kig2(t):  # [128,256] (ki32,n,g) -> [p,ki,n,d,g]
                return (kig2(t).unsqueeze(3)
                        .broadcast_to((128, 32, 2, 16, 4)))

            def TM5(t):  # [128,8192] cols (k,i,n,d,g) -> [p,ki64,n,d,g]
                return t.rearrange("p (ki n d g) -> p ki n d g", ki=64, n=2, d=16)

            def HM5(t):  # [128,4096] half tile -> [p,ki32,n,d,g]
                return t.rearrange("p (ki n d g) -> p ki n d g", ki=32, n=2, d=16)

            def kin3(t):  # [128,4096] half tile -> [p,k4,i,(ndg)]
                return t.rearrange("p (k i ndg) -> p k i ndg", k=4, i=8)

            def itree2(src3, out_kndg):
                """Sum over i of [p,k4,i,(ndg)] via tree-adds."""
                t1 = trees.tile([128, 2048], F16, tag="t2048", name="t2048")
                t13 = t1.rearrange("p (k i ndg) -> p k i ndg", k=4, i=4)
                nc.vector.tensor_tensor(out=t13, in0=src3[:, :, 0:4, :],
                                        in1=src3[:, :, 4:8, :], op=OP.add)
                t2 = trees.tile([128, 1024], F16, tag="t1024", name="t1024")
                t23 = t2.rearrange("p (k i ndg) -> p k i ndg", k=4, i=2)
                nc.vector.tensor_tensor(out=t23, in0=t13[:, :, 0:2, :],
                                        in1=t13[:, :, 2:4, :], op=OP.add)
                o3 = out_kndg.rearrange("p (k ndg) -> p k ndg", k=4).unsqueeze(2)
                nc.vector.tensor_tensor(out=o3, in0=t23[:, :, 0:1, :],
                                        in1=t23[:, :, 1:2, :], op=OP.add)

            def dtree2(src5, out_king):
                """Sum over d of [p,ki32,n,d,g] via tree-adds."""
                cur = src5
                nd = 16
                szs = {8: "t2048", 4: "t1024", 2: "t512"}
                while nd > 2:
                    nd //= 2
                    t = trees.tile([128, 32 * 2 * nd * 4], F16, tag=szs[nd],
                                   name=szs[nd])
                    t5 = t.rearrange("p (ki n d g) -> p ki n d g", ki=32, n=2,
                                     d=nd)
                    nc.vector.tensor_tensor(
                        out=t5, in0=cur[:, :, :, 0:nd, :],
                        in1=cur[:, :, :, nd:2 * nd, :], op=OP.add)
                    cur = t5
                o5 = kig2(out_king).unsqueeze(3)
                nc.vector.tensor_tensor(
                    out=o5, in0=cur[:, :, :, 0:1, :], in1=cur[:, :, :, 1:2, :],
                    op=OP.add)

            def split_tt(out5, in05, in15, op, cut):
                """One [p,ki32,n,d,g] TT, split: ki<cut on DVE, rest on Pool
                (concurrent engines, no serial coupling)."""
                if cut >= 32:
                    nc.vector.tensor_tensor(out=out5, in0=in05, in1=in15, op=op)
                    return
                nc.vector.tensor_tensor(
                    out=out5[:, 0:cut], in0=in05[:, 0:cut], in1=in15[:, 0:cut],
                    op=op)
                nc.gpsimd.tensor_tensor(
                    out=out5[:, cut:32], in0=in05[:, cut:32],
                    in1=in15[:, cut:32], op=op)

            def route_half(bl, h, cut_a, cut_b):
                T5 = TM5(T_tiles[bl])[:, h * 32:(h + 1) * 32]
                stM = stat_tiles[bl].rearrange(
                    "p (ki t n g) -> p ki t n g", ki=64, t=2, n=2)
                m1v = stM[:, h * 32:(h + 1) * 32, 0]   # [p,ki32,n,g]
                e2v = stM[:, h * 32:(h + 1) * 32, 1]

                # LayerNorm: var = E[t^2] - m1^2 ; rstd ; n2 = m1*rstd
                mm_ = sm.tile([128, 256], F16, tag="mm_", name="mm_")
                nc.vector.tensor_tensor(out=kig2(mm_), in0=m1v, in1=m1v,
                                        op=OP.mult)
                var = sm.tile([128, 256], F16, tag="var", name="var")
                nc.vector.tensor_tensor(out=kig2(var), in0=e2v, in1=kig2(mm_),
                                        op=OP.subtract)
                rstd = sm.tile([128, 256], F16, tag="rstd", name="rstd")
                nc.scalar.activation(rstd[:], var[:], AF.Sqrt, bias=eps5[:])
                nc.vector.reciprocal(rstd[:], rstd[:])
                n2 = sm.tile([128, 256], F16, tag="n2", name="n2")
                nc.vector.tensor_tensor(out=kig2(n2), in0=m1v, in1=kig2(rstd),
                                        op=OP.mult)

                # up = (T - m1) * rstd (optionally * gamma + beta)
                q = qp.tile([128, 4096], F16, tag="q", name="q")
                split_tt(HM5(q), T5, bc_kig2(rstd), OP.mult, cut_a)
                up = qp.tile([128, 4096], F16, tag="up", name="up")
                split_tt(HM5(up), HM5(q), bc_kig2(n2), OP.subtract, cut_a)
                if apply_gb:
                    up5 = HM5(up)
                    gb = (gam_sb[:].unsqueeze(1).unsqueeze(2).unsqueeze(4)
                          .broadcast_to((128, 32, 2, 16, 4)))
                    bb_ = (bet_sb[:].unsqueeze(1).unsqueeze(2).unsqueeze(4)
                           .broadcast_to((128, 32, 2, 16, 4)))
                    nc.vector.tensor_tensor(out=up5, in0=up5, in1=gb, op=OP.mult)
                    nc.vector.tensor_tensor(out=up5, in0=up5, in1=bb_, op=OP.add)

                # S = sum_i up ; dot_i = up_i . S
                S = sm.tile([128, 512], F16, tag="S", name="S")
                itree2(kin3(up), S)
                P = scr.tile([128, 4096], F16, tag="P", name="P")
                S_bc = (S.rearrange("p (k ndg) -> p k ndg", k=4).unsqueeze(2)
                        .broadcast_to((128, 4, 8, 128)))
                kc = cut_b // 8
                if kc >= 4:
                    nc.vector.tensor_tensor(out=kin3(P), in0=kin3(up),
                                            in1=S_bc, op=OP.mult)
                else:
                    nc.vector.tensor_tensor(
                        out=kin3(P)[:, 0:kc], in0=kin3(up)[:, 0:kc],
                        in1=S_bc[:, 0:kc], op=OP.mult)
                    nc.gpsimd.tensor_tensor(
                        out=kin3(P)[:, kc:4], in0=kin3(up)[:, kc:4],
                        in1=S_bc[:, kc:4], op=OP.mult)
                dot = sm.tile([128, 256], F16, tag="dot", name="dot")
                dtree2(HM5(P), dot)

                # rr_i = dot_i / max(||up_i||^2, 1e-8)
                rr = sm.tile([128, 256], F16, tag="rr", name="rr")
                if not apply_gb:
                    # 1/||up||^2 = (var+eps)/(16 var) = 1/16 + (eps/16)/var
                    ns_ = sm.tile([128, 256], F16, tag="ns_", name="ns_")
                    nc.vector.reciprocal(ns_[:], var[:])
                    nc.vector.tensor_scalar(
                        out=ns_[:], in0=ns_[:], scalar1=1e-5 / 16.0,
                        scalar2=1.0 / 16.0, op0=OP.mult, op1=OP.add)
                    nc.vector.tensor_tensor(out=rr[:], in0=dot[:], in1=ns_[:],
                                            op=OP.mult)
                else:
                    usq = scr.tile([128, 4096], F16, tag="P", name="usq")
                    nc.vector.tensor_tensor(out=usq[:], in0=up[:], in1=up[:],
                                            op=OP.mult)
                    nq = sm.tile([128, 256], F16, tag="nq", name="nq")
                    dtree2(HM5(usq), nq)
                    nc.vector.tensor_scalar_max(out=nq[:], in0=nq[:],
                                                scalar1=1e-8)
                    nc.vector.reciprocal(nq[:], nq[:])
                    nc.vector.tensor_tensor(out=rr[:], in0=dot[:], in1=nq[:],
                                            op=OP.mult)

                # softmax over i (shift-free: |rr| <= 8)
                es = sm.tile([128, 256], F16, tag="es", name="es")
                nc.scalar.activation(es[:], rr[:], AF.Exp)
                es4 = es.rearrange("p (k i n g) -> p k i n g", k=4, i=8, n=2)
                zt1 = trees.tile([128, 128], F16, tag="z4", name="z4")
                z14 = zt1.rearrange("p (k i n g) -> p k i n g", k=4, i=4, n=2)
                nc.vector.tensor_tensor(out=z14, in0=es4[:, :, 0:4],
                                        in1=es4[:, :, 4:8], op=OP.add)
                zt2 = trees.tile([128, 64], F16, tag="z2", name="z2")
                z24 = zt2.rearrange("p (k i n g) -> p k i n g", k=4, i=2, n=2)
                nc.vector.tensor_tensor(out=z24, in0=z14[:, :, 0:2],
                                        in1=z14[:, :, 2:4], op=OP.add)
                Z = sm.tile([128, 32], F16, tag="Z", name="Z")
                Z4 = Z.rearrange("p (k n g) -> p k n g", k=4, n=2).unsqueeze(2)
                nc.vector.tensor_tensor(out=Z4, in0=z24[:, :, 0:1],
                                        in1=z24[:, :, 1:2], op=OP.add)
                nc.vector.reciprocal(Z[:], Z[:])
                sc = sm.tile([128, 256], F16, tag="sc", name="sc")
                Zb = (Z.rearrange("p (k n g) -> p k n g", k=4, n=2).unsqueeze(2)
                      .broadcast_to((128, 4, 8, 2, 4)))
                sc4 = sc.rearrange("p (k i n g) -> p k i n g", k=4, i=8, n=2)
                nc.vector.tensor_tensor(out=sc4, in0=es4, in1=Zb, op=OP.mult)

                # s = sum_i score_i up_i ; squash over g
                P2 = scr.tile([128, 4096], F16, tag="P", name="P2")
                split_tt(HM5(P2), HM5(up), bc_kig2(sc), OP.mult, cut_b)
                s = sm.tile([128, 512], F16, tag="s", name="s")
                itree2(kin3(P2), s)
                ssq = sm.tile([128, 512], F16, tag="ssq", name="ssq")
                nc.vector.tensor_tensor(out=ssq[:], in0=s[:], in1=s[:], op=OP.mult)
                s4 = ssq.rearrange("p (knd g) -> p knd g", knd=128)
                gt = trees.tile([128, 256], F16, tag="g2", name="g2")
                gt4 = gt.rearrange("p (knd g) -> p knd g", knd=128)
                nc.vector.tensor_tensor(out=gt4, in0=s4[:, :, 0:2],
                                        in1=s4[:, :, 2:4], op=OP.add)
                nsq = sm.tile([128, 128], F16, tag="nsq", name="nsq")
                nsq4 = nsq[:].unsqueeze(2)
                nc.vector.tensor_tensor(out=nsq4, in0=gt4[:, :, 0:1],
                                        in1=gt4[:, :, 1:2], op=OP.add)
                sr = sm.tile([128, 128], F16, tag="sr", name="sr")
                nc.scalar.activation(sr[:], nsq[:], AF.Sqrt, bias=eps16[:])
                d1 = sm.tile([128, 128], F16, tag="d1", name="d1")
                nc.vector.scalar_tensor_tensor(
                    out=d1[:], in0=nsq[:], scalar=1.0, in1=sr[:],
                    op0=OP.add, op1=OP.mult)
                nc.vector.reciprocal(d1[:], d1[:])
                f = sm.tile([128, 128], F16, tag="f", name="f")
                nc.vector.tensor_tensor(out=f[:], in0=nsq[:], in1=d1[:], op=OP.mult)
                v = vp.tile([128, 512], F16, tag="v", name="v")
                v4 = v.rearrange("p (knd g) -> p knd g", knd=128)
                fb = f[:].unsqueeze(2).broadcast_to((128, 128, 4))
                s44 = s.rearrange("p (knd g) -> p knd g", knd=128)
                nc.vector.tensor_tensor(out=v4, in0=s44, in1=fb, op=OP.mult)
                u_ = bl * 1024 + h * 512
                nc.sync.dma_start(out=outd.ap()[:, u_:u_ + 512], in_=v[:])

            for bl in range(2):
                for chh in range(2):
                    conv_pass(bl, chh)
                    transS_half(bl, chh)
                    transT_half(bl, chh)
            route_half(0, 0, cut_a=32, cut_b=32)
            route_half(0, 1, cut_a=32, cut_b=24)
            route_half(1, 0, cut_a=24, cut_b=24)
            route_half(1, 1, cut_a=24, cut_b=24)

    _split_sync_waits(nc)
    return nc


def _pack_weights(conv_w):
    w = np.asarray(conv_w, np.float32)
    wt = np.stack(
        [np.roll(np.rot90(w, k=r, axes=(3, 4)), r, axis=2) for r in range(4)], axis=1
    )
    W512 = np.ascontiguousarray(wt.reshape(512, 64, 3, 3), dtype=np.float32)
    packs = []
    for pi in range(4):
        Wc = W512[128 * pi: 128 * pi + 128]  # 2 nouts' channels (n,d,g)
        w_pack = np.zeros((128, 6, 128), np.float32)
        for kx in range(3):
            w_pack[0:64, kx] = Wc[:, :, 0, kx].T
            w_pack[64:128, kx] = Wc[:, :, 1, kx].T
            w_pack[0:64, 3 + kx] = Wc[:, :, 2, kx].T
        packs.append(np.ascontiguousarray(
            w_pack.reshape(128, 768), dtype=np.float16))
    return packs


def _pack_caps(capsules):
    """[4,8,16,4,32,32] f32 -> [32,128,1164] f16 (padded image + row-shifted
    copy per [128]-partition tile)."""
    x = np.asarray(capsules, np.float32).reshape(32, 64, 32, 32)
    pad = np.zeros((32, 64, 34, 34), np.float16)
    pad[:, :, 1:33, 1:33] = x.astype(np.float16)
    A = pad.reshape(32, 64, 1156)
    buf = np.zeros((32, 128, 1164), np.float16)
    buf[:, 0:64, 0:1156] = A
    buf[:, 64:128, 0:1122] = A[:, :, 34:1156]
    return buf


_CACHE = {}


def kernel(capsules, conv_w, conv_b, ln_gamma, ln_beta):
    conv_b = np.asarray(conv_b, np.float32)
    ln_gamma = np.asarray(ln_gamma, np.float32)
    ln_beta = np.asarray(ln_beta, np.float32)
    apply_bias = bool(np.any(conv_b))
    apply_gb = bool(np.any(ln_gamma != 1.0) or np.any(ln_beta != 0.0))

    key = (apply_bias, apply_gb)
    if key not in _CACHE:
        _CACHE[key] = build_program(apply_bias=apply_bias, apply_gb=apply_gb)
    nc = _CACHE[key]

    capsd = _pack_caps(capsules)
    packs = _pack_weights(conv_w)
    identity = np.eye(128, dtype=np.float16)
    mmu = np.zeros((128, 16), np.float16)
    for ch in range(128):
        nn_, gg = ch // 64, ch % 4
        for t in range(2):
            mmu[ch, t * 8 + nn_ * 4 + gg] = 1.0 / 16.0
    in_maps = []
    for c in range(8):
        beta_ = c // 4   # batch-pair
        pi = c % 4       # nout-pair
        m = {"capsd": np.ascontiguousarray(capsd[16 * beta_: 16 * beta_ + 16]),
             "w": packs[pi], "ident": identity, "mmu": mmu}
        if apply_bias:
            # channel order (n,d,g): n*64 + d*4 + g
            b_loc = np.repeat(conv_b[32 * pi: 32 * pi + 32], 4).astype(np.float16)
            m["cb"] = np.ascontiguousarray(b_loc.reshape(1, 128))
        if apply_gb:
            m["gam"] = np.ascontiguousarray(ln_gamma.reshape(1, 16), dtype=np.float16)
            m["bet"] = np.ascontiguousarray(ln_beta.reshape(1, 16), dtype=np.float16)
        in_maps.append(m)

    res = run_bass_kernel_spmd(nc, in_maps, core_ids=list(range(8)), trace=False)
    # per-core out: [128, 2048] f16 = (p, bl, n, k, d, g); position = k*128+p
    out = np.zeros((4, 8, 16, 4, 32, 32), np.float32)
    for c in range(8):
        beta_, pi = c // 4, c % 4
        r = np.asarray(res.results[c]["outd"], np.float32).reshape(128, 2, 8, 2, 16, 4)
        for bl in range(2):
            for n in range(2):
                out[2 * beta_ + bl, 2 * pi + n] = (
                    r[:, bl, :, n].transpose(2, 3, 1, 0).reshape(16, 4, 32, 32))
    return out


# revision 38
# speedup vs baseline: 1.0193x; 1.0042x over previous
"""Trainium2 Bass kernel for nn_ConvolutionalCapsules.

Sharding: core c (of 8) owns output-capsules {2*(c%4), 2*(c%4)+1} for batches
{2*(c//4), 2*(c//4)+1}. Each core runs the p4 group conv with 128 output
channels (2 nout x 16 dout x 4 rot) over its 16 images (2 batches x 8 input
capsules), then LayerNorm + degree-score routing + squash per (batch, nout).

Conv: 3x3 conv as shifted matmuls from a zero-padded fp16 SBUF image (34x34
rows, dual-copy: partitions 0-63 = padded image, 64-127 = same shifted one row
so one K=128 matmul covers two filter taps). Weights are the stationary
operand ([K, 128 out-channels]), so each of the 6 matmuls per 512 positions
retires 2x the baseline's work. PE then transposes u ([128 chan, pos]) to the
routing layout ([128 pos, chan]) in fp16 (4x cheaper than f32 transposes).

Routing runs fully in fp16 on the DVE (tensor_tensor at the 2x perf mode,
tensor_scalar at 4x); segmented reductions over d/i/g are binary tree-adds of
strided views, which beat TensorReduce ~4x. LayerNorm is algebraically folded:
up = (T - mu)*rstd, rr_i = (up_i . S) * (var+eps)/(16 var), softmax over i
(shift-free: |rr| <= 8), s = sum_i score_i up_i, squash over g.

Host packs inputs (pad + dual-copy + fp16) and unpacks the [pos, (b,n,k,d,g)]
fp16 output, so every DMA moves contiguous >=512B lines.
"""

import numpy as np
from contextlib import ExitStack

import concourse.bass as bass
import concourse.tile as tile
from concourse import mybir
from concourse.bass_utils import run_bass_kernel_spmd

F16 = mybir.dt.float16
F32 = mybir.dt.float32
AF = mybir.ActivationFunctionType
OP = mybir.AluOpType

_ENGINES = {
    mybir.EngineType.PE,
    mybir.EngineType.Activation,
    mybir.EngineType.Pool,
    mybir.EngineType.DVE,
    mybir.EngineType.SP,
}


def _split_sync_waits(nc):
    """This walrus build accepts a single embedded sync-wait per instruction;
    hoist extras onto preceding NoOps on the same engine (ge-imm waits commute)."""
    for f in nc.m.functions:
        for bb in f.blocks:
            newl = []
            changed = False
            for inst in list(bb.instructions):
                si = inst.sync_info
                waits = list(si.on_wait) if si and si.on_wait else []
                if len(waits) > 1 and inst.engine in _ENGINES:
                    changed = True
                    for k, w in enumerate(waits[:-1]):
                        newl.append(
                            mybir.InstNoOp(
                                name=f"{inst.name}-ws{k}",
                                ins=[],
                                outs=[],
                                engine=inst.engine,
                                sync_info=mybir.SyncInfo(on_wait=[w], on_update=[]),
                            )
                        )
                    si.on_wait = waits[-1:]
                    inst.sync_info = si
                newl.append(inst)
            if changed:
                bb.instructions = newl


def build_program(apply_bias=False, apply_gb=False):
    nc = bass.Bass(trn_type="TRN2")
    capsd = nc.dram_tensor("capsd", [16, 128, 1164], F16, kind="ExternalInput")
    w = nc.dram_tensor("w", [128, 768], F16, kind="ExternalInput")
    ident = nc.dram_tensor("ident", [128, 128], F16, kind="ExternalInput")
    mmu = nc.dram_tensor("mmu", [128, 16], F16, kind="ExternalInput")
    if apply_bias:
        cb = nc.dram_tensor("cb", [1, 128], F16, kind="ExternalInput")
    if apply_gb:
        gam = nc.dram_tensor("gam", [1, 16], F16, kind="ExternalInput")
        bet = nc.dram_tensor("bet", [1, 16], F16, kind="ExternalInput")
    outd = nc.dram_tensor("outd", [128, 2048], F16, kind="ExternalOutput")

    with tile.TileContext(nc) as tc:
        with nc.allow_low_precision(reason="fp16 routing; 2e-2 rel tolerance"), \
             ExitStack() as ctx:
            consts = ctx.enter_context(tc.tile_pool(name="consts", bufs=1))
            imgs = ctx.enter_context(tc.tile_pool(name="imgs", bufs=1))
            us = ctx.enter_context(tc.tile_pool(name="us", bufs=1))
            ps = ctx.enter_context(tc.tile_pool(name="ps", bufs=4, space="PSUM"))
            tps = ctx.enter_context(tc.tile_pool(name="tps", bufs=3, space="PSUM"))
            sps = ctx.enter_context(tc.tile_pool(name="sps", bufs=1, space="PSUM"))
            Tp = ctx.enter_context(tc.tile_pool(name="Tp", bufs=1))
            qp = ctx.enter_context(tc.tile_pool(name="qp", bufs=1))
            scr = ctx.enter_context(tc.tile_pool(name="scr", bufs=2))
            trees = ctx.enter_context(tc.tile_pool(name="trees", bufs=2))
            sm = ctx.enter_context(tc.tile_pool(name="sm", bufs=2))
            vp = ctx.enter_context(tc.tile_pool(name="vp", bufs=2))

            w_sb = consts.tile([128, 768], F16, tag="w")
            nc.sync.dma_start(out=w_sb[:], in_=w.ap())
            id_sb = consts.tile([128, 128], F16, tag="ident")
            nc.sync.dma_start(out=id_sb[:], in_=ident.ap())
            mmu_sb = consts.tile([128, 16], F16, tag="mmu")
            nc.sync.dma_start(out=mmu_sb[:], in_=mmu.ap())
            eps5 = consts.tile([128, 1], F32, tag="eps5")
            nc.vector.memset(eps5[:], 1e-5)
            eps16 = consts.tile([128, 1], F32, tag="eps16")
            nc.vector.memset(eps16[:], 1e-16)
            if apply_bias:
                cb_sb = consts.tile([1, 128], F16, tag="cb")
                nc.sync.dma_start(out=cb_sb[:], in_=cb.ap())
                ones512 = consts.tile([1, 512], F16, tag="ones512")
                nc.vector.memset(ones512[:], 1.0)
            if apply_gb:
                gam_sb = consts.tile([128, 16], F16, tag="gam")
                nc.sync.dma_start(out=gam_sb[:], in_=gam.ap().partition_broadcast(128))
                bet_sb = consts.tile([128, 16], F16, tag="bet")
                nc.sync.dma_start(out=bet_sb[:], in_=bet.ap().partition_broadcast(128))

            def hview(ap_flat, o, rows):
                """[P, rows, 32] window at flat offset o, padded row stride 34."""
                return ap_flat[:, o: o + rows * 34].rearrange(
                    "c (h w) -> c h w", w=34
                )[:, :, 0:32]

            u_tiles = {}
            usq_tiles = {}
            T_tiles = {}
            stat_tiles = {}

            x_tiles = {}

            def conv_pass(bl, chh):
                """One 512-position chunk (4 k-blocks) of the conv for all 8
                images of batch bl. chh=0 covers k 0-3, chh=1 covers k 4-7."""
                base = chh * 16 * 34
                for i in range(8):
                    m = bl * 8 + i
                    if chh == 0:
                        xi = imgs.tile([128, 1164], F16, tag=f"x{i}", name=f"x{i}")
                        nc.sync.dma_start(out=xi[:], in_=capsd.ap()[m])
                        x_tiles[m] = xi
                        u = us.tile([128, 1024], F16, tag=f"u{i}", name=f"u{i}")
                        u_tiles[m] = u
                        usq = us.tile([128, 1024], F16, tag=f"usq{i}",
                                      name=f"usq{i}")
                        usq_tiles[m] = usq
                    xi, u, usq = x_tiles[m], u_tiles[m], usq_tiles[m]
                    p = ps.tile([128, 512], F32, tag="ps", name="ps")
                    for kx in range(3):
                        nc.tensor.matmul(
                            p[:],
                            lhsT=w_sb[:, kx * 128:(kx + 1) * 128],
                            rhs=hview(xi, base + kx, 16),
                            start=(kx == 0), stop=False)
                    for kx in range(3):
                        last = (kx == 2) and not apply_bias
                        nc.tensor.matmul(
                            p[:],
                            lhsT=w_sb[0:64, (3 + kx) * 128:(4 + kx) * 128],
                            rhs=hview(xi[0:64], base + 68 + kx, 16),
                            start=False, stop=last)
                    if apply_bias:
                        nc.tensor.matmul(
                            p[:], lhsT=cb_sb[:], rhs=ones512[:],
                            start=False, stop=True)
                    lo = chh * 512
                    nc.scalar.activation(u[:, lo:lo + 512], p[:], AF.Copy)
                    nc.gpsimd.tensor_tensor(
                        out=usq[:, lo:lo + 512], in0=u[:, lo:lo + 512],
                        in1=u[:, lo:lo + 512], op=OP.mult)

            def transS_half(bl, h):
                """LN stats (mean, mean-square over d) for k-blocks h*4..h*4+3
                on the PE via Mmu matmuls against u and u^2."""
                if h == 0:
                    stats = Tp.tile([128, 1024], F16, tag=f"st{bl}",
                                    name=f"st{bl}")
                    stat_tiles[bl] = stats
                stats = stat_tiles[bl]
                sp_ = sps.tile([128, 512], F32, tag="sps", name="sps")
                for kk in range(4):
                    k = h * 4 + kk
                    for j in range(8):
                        m = bl * 8 + j
                        co = kk * 128 + j * 16
                        nc.tensor.matmul(
                            sp_[:, co: co + 8],
                            lhsT=u_tiles[m][:, k * 128:(k + 1) * 128],
                            rhs=mmu_sb[:, 0:8], start=True, stop=True)
                        nc.tensor.matmul(
                            sp_[:, co + 8: co + 16],
                            lhsT=usq_tiles[m][:, k * 128:(k + 1) * 128],
                            rhs=mmu_sb[:, 8:16], start=True, stop=True)
                nc.scalar.activation(
                    stats[:, h * 512:(h + 1) * 512], sp_[:], AF.Copy)

            def transT_half(bl, h):
                """Transpose k-blocks h*4..h*4+3 of u into T[pos, (i,n,d,g)]."""
                if h == 0:
                    T = Tp.tile([128, 8192], F16, tag=f"T{bl}", name=f"T{bl}")
                    T_tiles[bl] = T
                T = T_tiles[bl]
                for kk in range(4):
                    k = h * 4 + kk
                    tp_ = tps.tile([128, 1024], F16, tag="tps", name="tps")
                    for j in range(8):
                        m = bl * 8 + j
                        nc.tensor.transpose(
                            out=tp_[:, j * 128:(j + 1) * 128],
                            in_=u_tiles[m][:, k * 128:(k + 1) * 128],
                            identity=id_sb[:])
                    nc.scalar.activation(
                        T[:, k * 1024:(k + 1) * 1024], tp_[:], AF.Copy)

            # ---- half-pair routing: both nout units of a batch for 4 of the
            # 8 position-blocks, as one set of double-width ops. Four such
            # chains pipeline against each other and the conv phases. ----

            def kig2(t):  # [128,256] cols (ki32, n, g)
                return t.rearrange("p (ki n g) -> p ki n g", ki=32, n=2)

            def bc_kig2(t):  # [128,256] (ki32,n,g) -> [p,ki,n,d,g]
                return (kig2(t).unsqueeze(3)
                        .broadcast_to((128, 32, 2, 16, 4)))

            def TM5(t):  # [128,8192] cols (k,i,n,d,g) -> [p,ki64,n,d,g]
                return t.rearrange("p (ki n d g) -> p ki n d g", ki=64, n=2, d=16)

            def HM5(t):  # [128,4096] half tile -> [p,ki32,n,d,g]
                return t.rearrange("p (ki n d g) -> p ki n d g", ki=32, n=2, d=16)

            def kin3(t):  # [128,4096] half tile -> [p,k4,i,(ndg)]
                return t.rearrange("p (k i ndg) -> p k i ndg", k=4, i=8)

            def itree2(src3, out_kndg):
                """Sum over i of [p,k4,i,(ndg)] via tree-adds."""
                t1 = trees.tile([128, 2048], F16, tag="t2048", name="t2048")
                t13 = t1.rearrange("p (k i ndg) -> p k i ndg", k=4, i=4)
                nc.vector.tensor_tensor(out=t13, in0=src3[:, :, 0:4, :],
                                        in1=src3[:, :, 4:8, :], op=OP.add)
                t2 = trees.tile([128, 1024], F16, tag="t1024", name="t1024")
                t23 = t2.rearrange("p (k i ndg) -> p k i ndg", k=4, i=2)
                nc.vector.tensor_tensor(out=t23, in0=t13[:, :, 0:2, :],
                                        in1=t13[:, :, 2:4, :], op=OP.add)
                o3 = out_kndg.rearrange("p (k ndg) -> p k ndg", k=4).unsqueeze(2)
                nc.vector.tensor_tensor(out=o3, in0=t23[:, :, 0:1, :],
                                        in1=t23[:, :, 1:2, :], op=OP.add)

            def dtree2(src5, out_king):
                """Sum over d of [p,ki32,n,d,g] via tree-adds."""
                cur = src5
                nd = 16
                szs = {8: "t2048", 4: "t1024", 2: "t512"}
                while nd > 2:
                    nd //= 2
                    t = trees.tile([128, 32 * 2 * nd * 4], F16, tag=szs[nd],
                                   name=szs[nd])
                    t5 = t.rearrange("p (ki n d g) -> p ki n d g", ki=32, n=2,
                                     d=nd)
                    nc.vector.tensor_tensor(
                        out=t5, in0=cur[:, :, :, 0:nd, :],
                        in1=cur[:, :, :, nd:2 * nd, :], op=OP.add)
                    cur = t5
                o5 = kig2(out_king).unsqueeze(3)
                nc.vector.tensor_tensor(
                    out=o5, in0=cur[:, :, :, 0:1, :], in1=cur[:, :, :, 1:2, :],
                    op=OP.add)

            def split_tt(out5, in05, in15, op, cut):
                """One [p,ki32,n,d,g] TT, split: ki<cut on DVE, rest on Pool
                (concurrent engines, no serial coupling)."""
                if cut >= 32:
                    nc.vector.tensor_tensor(out=out5, in0=in05, in1=in15, op=op)
                    return
                nc.vector.tensor_tensor(
                    out=out5[:, 0:cut], in0=in05[:, 0:cut], in1=in15[:, 0:cut],
                    op=op)
                nc.gpsimd.tensor_tensor(
                    out=out5[:, cut:32], in0=in05[:, cut:32],
                    in1=in15[:, cut:32], op=op)

            def route_half(bl, h, cut_a, cut_b):
                T5 = TM5(T_tiles[bl])[:, h * 32:(h + 1) * 32]
                stM = stat_tiles[bl].rearrange(
                    "p (ki t n g) -> p ki t n g", ki=64, t=2, n=2)
                m1v = stM[:, h * 32:(h + 1) * 32, 0]   # [p,ki32,n,g]
                e2v = stM[:, h * 32:(h + 1) * 32, 1]

                # LayerNorm: var = E[t^2] - m1^2 ; rstd ; n2 = m1*rstd
                mm_ = sm.tile([128, 256], F16, tag="mm_", name="mm_")
                nc.scalar.activation(kig2(mm_), m1v, AF.Square)
                var = sm.tile([128, 256], F16, tag="var", name="var")
                nc.vector.tensor_tensor(out=kig2(var), in0=e2v, in1=kig2(mm_),
                                        op=OP.subtract)
                rstd = sm.tile([128, 256], F16, tag="rstd", name="rstd")
                nc.scalar.activation(rstd[:], var[:], AF.Sqrt, bias=eps5[:])
                nc.vector.reciprocal(rstd[:], rstd[:])
                n2 = sm.tile([128, 256], F16, tag="n2", name="n2")
                nc.vector.tensor_tensor(out=kig2(n2), in0=m1v, in1=kig2(rstd),
                                        op=OP.mult)

                # up = (T - m1) * rstd (optionally * gamma + beta)
                q = qp.tile([128, 4096], F16, tag="q", name="q")
                split_tt(HM5(q), T5, bc_kig2(rstd), OP.mult, cut_a)
                up = qp.tile([128, 4096], F16, tag="up", name="up")
                split_tt(HM5(up), HM5(q), bc_kig2(n2), OP.subtract, cut_a)
                if apply_gb:
                    up5 = HM5(up)
                    gb = (gam_sb[:].unsqueeze(1).unsqueeze(2).unsqueeze(4)
                          .broadcast_to((128, 32, 2, 16, 4)))
                    bb_ = (bet_sb[:].unsqueeze(1).unsqueeze(2).unsqueeze(4)
                           .broadcast_to((128, 32, 2, 16, 4)))
                    nc.vector.tensor_tensor(out=up5, in0=up5, in1=gb, op=OP.mult)
                    nc.vector.tensor_tensor(out=up5, in0=up5, in1=bb_, op=OP.add)

                # S = sum_i up ; dot_i = up_i . S
                S = sm.tile([128, 512], F16, tag="S", name="S")
                itree2(kin3(up), S)
                P = scr.tile([128, 4096], F16, tag="P", name="P")
                S_bc = (S.rearrange("p (k ndg) -> p k ndg", k=4).unsqueeze(2)
                        .broadcast_to((128, 4, 8, 128)))
                kc = cut_b // 8
                if kc >= 4:
                    nc.vector.tensor_tensor(out=kin3(P), in0=kin3(up),
                                            in1=S_bc, op=OP.mult)
                else:
                    nc.vector.tensor_tensor(
                        out=kin3(P)[:, 0:kc], in0=kin3(up)[:, 0:kc],
                        in1=S_bc[:, 0:kc], op=OP.mult)
                    nc.gpsimd.tensor_tensor(
                        out=kin3(P)[:, kc:4], in0=kin3(up)[:, kc:4],
                        in1=S_bc[:, kc:4], op=OP.mult)
                dot = sm.tile([128, 256], F16, tag="dot", name="dot")
                dtree2(HM5(P), dot)

                # rr_i = dot_i / max(||up_i||^2, 1e-8)
                rr = sm.tile([128, 256], F16, tag="rr", name="rr")
                if not apply_gb:
                    # 1/||up||^2 = (var+eps)/(16 var) = 1/16 + (eps/16)/var
                    ns_ = sm.tile([128, 256], F16, tag="ns_", name="ns_")
                    nc.vector.reciprocal(ns_[:], var[:])
                    nc.vector.tensor_scalar(
                        out=ns_[:], in0=ns_[:], scalar1=1e-5 / 16.0,
                        scalar2=1.0 / 16.0, op0=OP.mult, op1=OP.add)
                    nc.vector.tensor_tensor(out=rr[:], in0=dot[:], in1=ns_[:],
                                            op=OP.mult)
                else:
                    usq = scr.tile([128, 4096], F16, tag="P", name="usq")
                    nc.vector.tensor_tensor(out=usq[:], in0=up[:], in1=up[:],
                                            op=OP.mult)
                    nq = sm.tile([128, 256], F16, tag="nq", name="nq")
                    dtree2(HM5(usq), nq)
                    nc.vector.tensor_scalar_max(out=nq[:], in0=nq[:],
                                                scalar1=1e-8)
                    nc.vector.reciprocal(nq[:], nq[:])
                    nc.vector.tensor_tensor(out=rr[:], in0=dot[:], in1=nq[:],
                                            op=OP.mult)

                # softmax over i (shift-free: |rr| <= 8)
                es = sm.tile([128, 256], F16, tag="es", name="es")
                nc.scalar.activation(es[:], rr[:], AF.Exp)
                es4 = es.rearrange("p (k i n g) -> p k i n g", k=4, i=8, n=2)
                zt1 = trees.tile([128, 128], F16, tag="z4", name="z4")
                z14 = zt1.rearrange("p (k i n g) -> p k i n g", k=4, i=4, n=2)
                nc.vector.tensor_tensor(out=z14, in0=es4[:, :, 0:4],
                                        in1=es4[:, :, 4:8], op=OP.add)
                zt2 = trees.tile([128, 64], F16, tag="z2", name="z2")
                z24 = zt2.rearrange("p (k i n g) -> p k i n g", k=4, i=2, n=2)
                nc.vector.tensor_tensor(out=z24, in0=z14[:, :, 0:2],
                                        in1=z14[:, :, 2:4], op=OP.add)
                Z = sm.tile([128, 32], F16, tag="Z", name="Z")
                Z4 = Z.rearrange("p (k n g) -> p k n g", k=4, n=2).unsqueeze(2)
                nc.vector.tensor_tensor(out=Z4, in0=z24[:, :, 0:1],
                                        in1=z24[:, :, 1:2], op=OP.add)
                nc.vector.reciprocal(Z[:], Z[:])
                sc = sm.tile([128, 256], F16, tag="sc", name="sc")
                Zb = (Z.rearrange("p (k n g) -> p k n g", k=4, n=2).unsqueeze(2)
                      .broadcast_to((128, 4, 8, 2, 4)))
                sc4 = sc.rearrange("p (k i n g) -> p k i n g", k=4, i=8, n=2)
                nc.vector.tensor_tensor(out=sc4, in0=es4, in1=Zb, op=OP.mult)

                # s = sum_i score_i up_i ; squash over g
                P2 = scr.tile([128, 4096], F16, tag="P", name="P2")
                split_tt(HM5(P2), HM5(up), bc_kig2(sc), OP.mult, cut_b)
                s = sm.tile([128, 512], F16, tag="s", name="s")
                itree2(kin3(P2), s)
                ssq = sm.tile([128, 512], F16, tag="ssq", name="ssq")
                nc.scalar.activation(ssq[:], s[:], AF.Square)
                s4 = ssq.rearrange("p (knd g) -> p knd g", knd=128)
                gt = trees.tile([128, 256], F16, tag="g2", name="g2")
                gt4 = gt.rearrange("p (knd g) -> p knd g", knd=128)
                nc.vector.tensor_tensor(out=gt4, in0=s4[:, :, 0:2],
                                        in1=s4[:, :, 2:4], op=OP.add)
                nsq = sm.tile([128, 128], F16, tag="nsq", name="nsq")
                nsq4 = nsq[:].unsqueeze(2)
                nc.vector.tensor_tensor(out=nsq4, in0=gt4[:, :, 0:1],
                                        in1=gt4[:, :, 1:2], op=OP.add)
                sr = sm.tile([128, 128], F16, tag="sr", name="sr")
                nc.scalar.activation(sr[:], nsq[:], AF.Sqrt, bias=eps16[:])
                d1 = sm.tile([128, 128], F16, tag="d1", name="d1")
                nc.vector.scalar_tensor_tensor(
                    out=d1[:], in0=nsq[:], scalar=1.0, in1=sr[:],
                    op0=OP.add, op1=OP.mult)
                nc.vector.reciprocal(d1[:], d1[:])
                f = sm.tile([128, 128], F16, tag="f", name="f")
                nc.vector.tensor_tensor(out=f[:], in0=nsq[:], in1=d1[:], op=OP.mult)
                v = vp.tile([128, 512], F16, tag="v", name="v")
                v4 = v.rearrange("p (knd g) -> p knd g", knd=128)
                fb = f[:].unsqueeze(2).broadcast_to((128, 128, 4))
                s44 = s.rearrange("p (knd g) -> p knd g", knd=128)
                nc.vector.tensor_tensor(out=v4, in0=s44, in1=fb, op=OP.mult)
                u_ = bl * 1024 + h * 512
                nc.sync.dma_start(out=outd.ap()[:, u_:u_ + 512], in_=v[:])

            for bl in range(2):
                for chh in range(2):
                    conv_pass(bl, chh)
                    transS_half(bl, chh)
                    transT_half(bl, chh)
            route_half(0, 0, cut_a=32, cut_b=32)
            route_half(0, 1, cut_a=32, cut_b=24)
            route_half(1, 0, cut_a=24, cut_b=24)
            route_half(1, 1, cut_a=24, cut_b=24)

    _split_sync_waits(nc)
    return nc


def _pack_weights(conv_w):
    w = np.asarray(conv_w, np.float32)
    wt = np.stack(
        [np.roll(np.rot90(w, k=r, axes=(3, 4)), r, axis=2) for r in range(4)], axis=1
    )
    W512 = np.ascontiguousarray(wt.reshape(512, 64, 3, 3), dtype=np.float32)
    packs = []
    for pi in range(4):
        Wc = W512[128 * pi: 128 * pi + 128]  # 2 nouts' channels (n,d,g)
        w_pack = np.zeros((128, 6, 128), np.float32)
        for kx in range(3):
            w_pack[0:64, kx] = Wc[:, :, 0, kx].T
            w_pack[64:128, kx] = Wc[:, :, 1, kx].T
            w_pack[0:64, 3 + kx] = Wc[:, :, 2, kx].T
        packs.append(np.ascontiguousarray(
            w_pack.reshape(128, 768), dtype=np.float16))
    return packs


def _pack_caps(capsules):
    """[4,8,16,4,32,32] f32 -> [32,128,1164] f16 (padded image + row-shifted
    copy per [128]-partition tile)."""
    x = np.asarray(capsules, np.float32).reshape(32, 64, 32, 32)
    pad = np.zeros((32, 64, 34, 34), np.float16)
    pad[:, :, 1:33, 1:33] = x.astype(np.float16)
    A = pad.reshape(32, 64, 1156)
    buf = np.zeros((32, 128, 1164), np.float16)
    buf[:, 0:64, 0:1156] = A
    buf[:, 64:128, 0:1122] = A[:, :, 34:1156]
    return buf


_CACHE = {}


def kernel(capsules, conv_w, conv_b, ln_gamma, ln_beta):
    conv_b = np.asarray(conv_b, np.float32)
    ln_gamma = np.asarray(ln_gamma, np.float32)
    ln_beta = np.asarray(ln_beta, np.float32)
    apply_bias = bool(np.any(conv_b))
    apply_gb = bool(np.any(ln_gamma != 1.0) or np.any(ln_beta != 0.0))

    key = (apply_bias, apply_gb)
    if key not in _CACHE:
        _CACHE[key] = build_program(apply_bias=apply_bias, apply_gb=apply_gb)
    nc = _CACHE[key]

    capsd = _pack_caps(capsules)
    packs = _pack_weights(conv_w)
    identity = np.eye(128, dtype=np.float16)
    mmu = np.zeros((128, 16), np.float16)
    for ch in range(128):
        nn_, gg = ch // 64, ch % 4
        for t in range(2):
            mmu[ch, t * 8 + nn_ * 4 + gg] = 1.0 / 16.0
    in_maps = []
    for c in range(8):
        beta_ = c // 4   # batch-pair
        pi = c % 4       # nout-pair
        m = {"capsd": np.ascontiguousarray(capsd[16 * beta_: 16 * beta_ + 16]),
             "w": packs[pi], "ident": identity, "mmu": mmu}
        if apply_bias:
            # channel order (n,d,g): n*64 + d*4 + g
            b_loc = np.repeat(conv_b[32 * pi: 32 * pi + 32], 4).astype(np.float16)
            m["cb"] = np.ascontiguousarray(b_loc.reshape(1, 128))
        if apply_gb:
            m["gam"] = np.ascontiguousarray(ln_gamma.reshape(1, 16), dtype=np.float16)
            m["bet"] = np.ascontiguousarray(ln_beta.reshape(1, 16), dtype=np.float16)
        in_maps.append(m)

    res = run_bass_kernel_spmd(nc, in_maps, core_ids=list(range(8)), trace=False)
    # per-core out: [128, 2048] f16 = (p, bl, n, k, d, g); position = k*128+p
    out = np.zeros((4, 8, 16, 4, 32, 32), np.float32)
    for c in range(8):
        beta_, pi = c // 4, c % 4
        r = np.asarray(res.results[c]["outd"], np.float32).reshape(128, 2, 8, 2, 16, 4)
        for bl in range(2):
            for n in range(2):
                out[2 * beta_ + bl, 2 * pi + n] = (
                    r[:, bl, :, n].transpose(2, 3, 1, 0).reshape(16, 4, 32, 32))
    return out


# revision 39
# speedup vs baseline: 1.0285x; 1.0090x over previous
"""Trainium2 Bass kernel for nn_ConvolutionalCapsules.

Sharding: core c (of 8) owns output-capsules {2*(c%4), 2*(c%4)+1} for batches
{2*(c//4), 2*(c//4)+1}. Each core runs the p4 group conv with 128 output
channels (2 nout x 16 dout x 4 rot) over its 16 images (2 batches x 8 input
capsules), then LayerNorm + degree-score routing + squash per (batch, nout).

Conv: 3x3 conv as shifted matmuls from a zero-padded fp16 SBUF image (34x34
rows, dual-copy: partitions 0-63 = padded image, 64-127 = same shifted one row
so one K=128 matmul covers two filter taps). Weights are the stationary
operand ([K, 128 out-channels]), so each of the 6 matmuls per 512 positions
retires 2x the baseline's work. PE then transposes u ([128 chan, pos]) to the
routing layout ([128 pos, chan]) in fp16 (4x cheaper than f32 transposes).

Routing runs fully in fp16 on the DVE (tensor_tensor at the 2x perf mode,
tensor_scalar at 4x); segmented reductions over d/i/g are binary tree-adds of
strided views, which beat TensorReduce ~4x. LayerNorm is algebraically folded:
up = (T - mu)*rstd, rr_i = (up_i . S) * (var+eps)/(16 var), softmax over i
(shift-free: |rr| <= 8), s = sum_i score_i up_i, squash over g.

Host packs inputs (pad + dual-copy + fp16) and unpacks the [pos, (b,n,k,d,g)]
fp16 output, so every DMA moves contiguous >=512B lines.
"""

import numpy as np
from contextlib import ExitStack

import concourse.bass as bass
import concourse.tile as tile
from concourse import mybir
from concourse.bass_utils import run_bass_kernel_spmd

F16 = mybir.dt.float16
F32 = mybir.dt.float32
AF = mybir.ActivationFunctionType
OP = mybir.AluOpType

_ENGINES = {
    mybir.EngineType.PE,
    mybir.EngineType.Activation,
    mybir.EngineType.Pool,
    mybir.EngineType.DVE,
    mybir.EngineType.SP,
}


def _split_sync_waits(nc):
    """This walrus build accepts a single embedded sync-wait per instruction;
    hoist extras onto preceding NoOps on the same engine (ge-imm waits commute)."""
    for f in nc.m.functions:
        for bb in f.blocks:
            newl = []
            changed = False
            for inst in list(bb.instructions):
                si = inst.sync_info
                waits = list(si.on_wait) if si and si.on_wait else []
                if len(waits) > 1 and inst.engine in _ENGINES:
                    changed = True
                    for k, w in enumerate(waits[:-1]):
                        newl.append(
                            mybir.InstNoOp(
                                name=f"{inst.name}-ws{k}",
                                ins=[],
                                outs=[],
                                engine=inst.engine,
                                sync_info=mybir.SyncInfo(on_wait=[w], on_update=[]),
                            )
                        )
                    si.on_wait = waits[-1:]
                    inst.sync_info = si
                newl.append(inst)
            if changed:
                bb.instructions = newl


def build_program(apply_bias=False, apply_gb=False):
    nc = bass.Bass(trn_type="TRN2")
    capsd = nc.dram_tensor("capsd", [16, 128, 1164], F16, kind="ExternalInput")
    w = nc.dram_tensor("w", [128, 768], F16, kind="ExternalInput")
    ident = nc.dram_tensor("ident", [128, 128], F16, kind="ExternalInput")
    mmu = nc.dram_tensor("mmu", [128, 16], F16, kind="ExternalInput")
    if apply_bias:
        cb = nc.dram_tensor("cb", [1, 128], F16, kind="ExternalInput")
    if apply_gb:
        gam = nc.dram_tensor("gam", [1, 16], F16, kind="ExternalInput")
        bet = nc.dram_tensor("bet", [1, 16], F16, kind="ExternalInput")
    outd = nc.dram_tensor("outd", [128, 2048], F16, kind="ExternalOutput")

    with tile.TileContext(nc) as tc:
        with nc.allow_low_precision(reason="fp16 routing; 2e-2 rel tolerance"), \
             ExitStack() as ctx:
            consts = ctx.enter_context(tc.tile_pool(name="consts", bufs=1))
            imgs = ctx.enter_context(tc.tile_pool(name="imgs", bufs=1))
            us = ctx.enter_context(tc.tile_pool(name="us", bufs=1))
            ps = ctx.enter_context(tc.tile_pool(name="ps", bufs=4, space="PSUM"))
            tps = ctx.enter_context(tc.tile_pool(name="tps", bufs=3, space="PSUM"))
            sps = ctx.enter_context(tc.tile_pool(name="sps", bufs=1, space="PSUM"))
            Tp = ctx.enter_context(tc.tile_pool(name="Tp", bufs=1))
            qp = ctx.enter_context(tc.tile_pool(name="qp", bufs=2))
            scr = ctx.enter_context(tc.tile_pool(name="scr", bufs=3))
            trees = ctx.enter_context(tc.tile_pool(name="trees", bufs=2))
            sm = ctx.enter_context(tc.tile_pool(name="sm", bufs=3))
            vp = ctx.enter_context(tc.tile_pool(name="vp", bufs=2))

            w_sb = consts.tile([128, 768], F16, tag="w")
            nc.sync.dma_start(out=w_sb[:], in_=w.ap())
            id_sb = consts.tile([128, 128], F16, tag="ident")
            nc.sync.dma_start(out=id_sb[:], in_=ident.ap())
            mmu_sb = consts.tile([128, 16], F16, tag="mmu")
            nc.sync.dma_start(out=mmu_sb[:], in_=mmu.ap())
            eps5 = consts.tile([128, 1], F32, tag="eps5")
            nc.vector.memset(eps5[:], 1e-5)
            eps16 = consts.tile([128, 1], F32, tag="eps16")
            nc.vector.memset(eps16[:], 1e-16)
            if apply_bias:
                cb_sb = consts.tile([1, 128], F16, tag="cb")
                nc.sync.dma_start(out=cb_sb[:], in_=cb.ap())
                ones512 = consts.tile([1, 512], F16, tag="ones512")
                nc.vector.memset(ones512[:], 1.0)
            if apply_gb:
                gam_sb = consts.tile([128, 16], F16, tag="gam")
                nc.sync.dma_start(out=gam_sb[:], in_=gam.ap().partition_broadcast(128))
                bet_sb = consts.tile([128, 16], F16, tag="bet")
                nc.sync.dma_start(out=bet_sb[:], in_=bet.ap().partition_broadcast(128))

            def hview(ap_flat, o, rows):
                """[P, rows, 32] window at flat offset o, padded row stride 34."""
                return ap_flat[:, o: o + rows * 34].rearrange(
                    "c (h w) -> c h w", w=34
                )[:, :, 0:32]

            u_tiles = {}
            usq_tiles = {}
            T_tiles = {}
            stat_tiles = {}

            x_tiles = {}

            def conv_pass(bl, chh):
                """One 512-position chunk (4 k-blocks) of the conv for all 8
                images of batch bl. chh=0 covers k 0-3, chh=1 covers k 4-7."""
                base = chh * 16 * 34
                for i in range(8):
                    m = bl * 8 + i
                    if chh == 0:
                        xi = imgs.tile([128, 1164], F16, tag=f"x{i}", name=f"x{i}")
                        nc.sync.dma_start(out=xi[:], in_=capsd.ap()[m])
                        x_tiles[m] = xi
                        u = us.tile([128, 1024], F16, tag=f"u{i}", name=f"u{i}")
                        u_tiles[m] = u
                        usq = us.tile([128, 1024], F16, tag=f"usq{i}",
                                      name=f"usq{i}")
                        usq_tiles[m] = usq
                    xi, u, usq = x_tiles[m], u_tiles[m], usq_tiles[m]
                    p = ps.tile([128, 512], F32, tag="ps", name="ps")
                    for kx in range(3):
                        nc.tensor.matmul(
                            p[:],
                            lhsT=w_sb[:, kx * 128:(kx + 1) * 128],
                            rhs=hview(xi, base + kx, 16),
                            start=(kx == 0), stop=False)
                    for kx in range(3):
                        last = (kx == 2) and not apply_bias
                        nc.tensor.matmul(
                            p[:],
                            lhsT=w_sb[0:64, (3 + kx) * 128:(4 + kx) * 128],
                            rhs=hview(xi[0:64], base + 68 + kx, 16),
                            start=False, stop=last)
                    if apply_bias:
                        nc.tensor.matmul(
                            p[:], lhsT=cb_sb[:], rhs=ones512[:],
                            start=False, stop=True)
                    lo = chh * 512
                    nc.scalar.activation(u[:, lo:lo + 512], p[:], AF.Copy)
                    nc.gpsimd.tensor_tensor(
                        out=usq[:, lo:lo + 512], in0=u[:, lo:lo + 512],
                        in1=u[:, lo:lo + 512], op=OP.mult)

            def transS_half(bl, h):
                """LN stats (mean, mean-square over d) for k-blocks h*4..h*4+3
                on the PE via Mmu matmuls against u and u^2."""
                if h == 0:
                    stats = Tp.tile([128, 1024], F16, tag=f"st{bl}",
                                    name=f"st{bl}")
                    stat_tiles[bl] = stats
                stats = stat_tiles[bl]
                sp_ = sps.tile([128, 512], F32, tag="sps", name="sps")
                for kk in range(4):
                    k = h * 4 + kk
                    for j in range(8):
                        m = bl * 8 + j
                        co = kk * 128 + j * 16
                        nc.tensor.matmul(
                            sp_[:, co: co + 8],
                            lhsT=u_tiles[m][:, k * 128:(k + 1) * 128],
                            rhs=mmu_sb[:, 0:8], start=True, stop=True)
                        nc.tensor.matmul(
                            sp_[:, co + 8: co + 16],
                            lhsT=usq_tiles[m][:, k * 128:(k + 1) * 128],
                            rhs=mmu_sb[:, 8:16], start=True, stop=True)
                nc.scalar.activation(
                    stats[:, h * 512:(h + 1) * 512], sp_[:], AF.Copy)

            def transT_half(bl, h):
                """Transpose k-blocks h*4..h*4+3 of u into T[pos, (i,n,d,g)]."""
                if h == 0:
                    T = Tp.tile([128, 8192], F16, tag=f"T{bl}", name=f"T{bl}")
                    T_tiles[bl] = T
                T = T_tiles[bl]
                for kk in range(4):
                    k = h * 4 + kk
                    tp_ = tps.tile([128, 1024], F16, tag="tps", name="tps")
                    for j in range(8):
                        m = bl * 8 + j
                        nc.tensor.transpose(
                            out=tp_[:, j * 128:(j + 1) * 128],
                            in_=u_tiles[m][:, k * 128:(k + 1) * 128],
                            identity=id_sb[:])
                    nc.scalar.activation(
                        T[:, k * 1024:(k + 1) * 1024], tp_[:], AF.Copy)

            # ---- half-pair routing: both nout units of a batch for 4 of the
            # 8 position-blocks, as one set of double-width ops. Four such
            # chains pipeline against each other and the conv phases. ----

            def kig2(t):  # [128,256] cols (ki32, n, g)
                return t.rearrange("p (ki n g) -> p ki n g", ki=32, n=2)

            def bc_kig2(t):  # [128,256] (ki32,n,g) -> [p,ki,n,d,g]
                return (kig2(t).unsqueeze(3)
                        .broadcast_to((128, 32, 2, 16, 4)))

            def TM5(t):  # [128,8192] cols (k,i,n,d,g) -> [p,ki64,n,d,g]
                return t.rearrange("p (ki n d g) -> p ki n d g", ki=64, n=2, d=16)

            def HM5(t):  # [128,4096] half tile -> [p,ki32,n,d,g]
                return t.rearrange("p (ki n d g) -> p ki n d g", ki=32, n=2, d=16)

            def kin3(t):  # [128,4096] half tile -> [p,k4,i,(ndg)]
                return t.rearrange("p (k i ndg) -> p k i ndg", k=4, i=8)

            def itree2(src3, out_kndg):
                """Sum over i of [p,k4,i,(ndg)] via tree-adds."""
                t1 = trees.tile([128, 2048], F16, tag="t2048", name="t2048")
                t13 = t1.rearrange("p (k i ndg) -> p k i ndg", k=4, i=4)
                nc.vector.tensor_tensor(out=t13, in0=src3[:, :, 0:4, :],
                                        in1=src3[:, :, 4:8, :], op=OP.add)
                t2 = trees.tile([128, 1024], F16, tag="t1024", name="t1024")
                t23 = t2.rearrange("p (k i ndg) -> p k i ndg", k=4, i=2)
                nc.vector.tensor_tensor(out=t23, in0=t13[:, :, 0:2, :],
                                        in1=t13[:, :, 2:4, :], op=OP.add)
                o3 = out_kndg.rearrange("p (k ndg) -> p k ndg", k=4).unsqueeze(2)
                nc.vector.tensor_tensor(out=o3, in0=t23[:, :, 0:1, :],
                                        in1=t23[:, :, 1:2, :], op=OP.add)

            def dtree2(src5, out_king):
                """Sum over d of [p,ki32,n,d,g] via tree-adds."""
                cur = src5
                nd = 16
                szs = {8: "t2048", 4: "t1024", 2: "t512"}
                while nd > 2:
                    nd //= 2
                    t = trees.tile([128, 32 * 2 * nd * 4], F16, tag=szs[nd],
                                   name=szs[nd])
                    t5 = t.rearrange("p (ki n d g) -> p ki n d g", ki=32, n=2,
                                     d=nd)
                    nc.vector.tensor_tensor(
                        out=t5, in0=cur[:, :, :, 0:nd, :],
                        in1=cur[:, :, :, nd:2 * nd, :], op=OP.add)
                    cur = t5
                o5 = kig2(out_king).unsqueeze(3)
                nc.vector.tensor_tensor(
                    out=o5, in0=cur[:, :, :, 0:1, :], in1=cur[:, :, :, 1:2, :],
                    op=OP.add)

            def split_tt(out5, in05, in15, op, cut):
                """One [p,ki32,n,d,g] TT, split: ki<cut on DVE, rest on Pool
                (concurrent engines, no serial coupling)."""
                if cut >= 32:
                    nc.vector.tensor_tensor(out=out5, in0=in05, in1=in15, op=op)
                    return
                nc.vector.tensor_tensor(
                    out=out5[:, 0:cut], in0=in05[:, 0:cut], in1=in15[:, 0:cut],
                    op=op)
                nc.gpsimd.tensor_tensor(
                    out=out5[:, cut:32], in0=in05[:, cut:32],
                    in1=in15[:, cut:32], op=op)

            def route_half(bl, h, cut_a, cut_b):
                T5 = TM5(T_tiles[bl])[:, h * 32:(h + 1) * 32]
                stM = stat_tiles[bl].rearrange(
                    "p (ki t n g) -> p ki t n g", ki=64, t=2, n=2)
                m1v = stM[:, h * 32:(h + 1) * 32, 0]   # [p,ki32,n,g]
                e2v = stM[:, h * 32:(h + 1) * 32, 1]

                # LayerNorm: var = E[t^2] - m1^2 ; rstd ; n2 = m1*rstd
                mm_ = sm.tile([128, 256], F16, tag="mm_", name="mm_")
                nc.scalar.activation(kig2(mm_), m1v, AF.Square)
                var = sm.tile([128, 256], F16, tag="var", name="var")
                nc.vector.tensor_tensor(out=kig2(var), in0=e2v, in1=kig2(mm_),
                                        op=OP.subtract)
                rstd = sm.tile([128, 256], F16, tag="rstd", name="rstd")
                nc.scalar.activation(rstd[:], var[:], AF.Sqrt, bias=eps5[:])
                nc.vector.reciprocal(rstd[:], rstd[:])
                n2 = sm.tile([128, 256], F16, tag="n2", name="n2")
                nc.vector.tensor_tensor(out=kig2(n2), in0=m1v, in1=kig2(rstd),
                                        op=OP.mult)

                # up = (T - m1) * rstd (optionally * gamma + beta)
                q = qp.tile([128, 4096], F16, tag="q", name="q")
                split_tt(HM5(q), T5, bc_kig2(rstd), OP.mult, cut_a)
                up = qp.tile([128, 4096], F16, tag="up", name="up")
                split_tt(HM5(up), HM5(q), bc_kig2(n2), OP.subtract, cut_a)
                if apply_gb:
                    up5 = HM5(up)
                    gb = (gam_sb[:].unsqueeze(1).unsqueeze(2).unsqueeze(4)
                          .broadcast_to((128, 32, 2, 16, 4)))
                    bb_ = (bet_sb[:].unsqueeze(1).unsqueeze(2).unsqueeze(4)
                           .broadcast_to((128, 32, 2, 16, 4)))
                    nc.vector.tensor_tensor(out=up5, in0=up5, in1=gb, op=OP.mult)
                    nc.vector.tensor_tensor(out=up5, in0=up5, in1=bb_, op=OP.add)

                # S = sum_i up ; dot_i = up_i . S
                S = sm.tile([128, 512], F16, tag="S", name="S")
                itree2(kin3(up), S)
                P = scr.tile([128, 4096], F16, tag="P", name="P")
                S_bc = (S.rearrange("p (k ndg) -> p k ndg", k=4).unsqueeze(2)
                        .broadcast_to((128, 4, 8, 128)))
                kc = cut_b // 8
                if kc >= 4:
                    nc.vector.tensor_tensor(out=kin3(P), in0=kin3(up),
                                            in1=S_bc, op=OP.mult)
                else:
                    nc.vector.tensor_tensor(
                        out=kin3(P)[:, 0:kc], in0=kin3(up)[:, 0:kc],
                        in1=S_bc[:, 0:kc], op=OP.mult)
                    nc.gpsimd.tensor_tensor(
                        out=kin3(P)[:, kc:4], in0=kin3(up)[:, kc:4],
                        in1=S_bc[:, kc:4], op=OP.mult)
                dot = sm.tile([128, 256], F16, tag="dot", name="dot")
                dtree2(HM5(P), dot)

                # rr_i = dot_i / max(||up_i||^2, 1e-8)
                rr = sm.tile([128, 256], F16, tag="rr", name="rr")
                if not apply_gb:
                    # 1/||up||^2 = (var+eps)/(16 var) = 1/16 + (eps/16)/var
                    ns_ = sm.tile([128, 256], F16, tag="ns_", name="ns_")
                    nc.vector.reciprocal(ns_[:], var[:])
                    nc.vector.tensor_scalar(
                        out=ns_[:], in0=ns_[:], scalar1=1e-5 / 16.0,
                        scalar2=1.0 / 16.0, op0=OP.mult, op1=OP.add)
                    nc.vector.tensor_tensor(out=rr[:], in0=dot[:], in1=ns_[:],
                                            op=OP.mult)
                else:
                    usq = scr.tile([128, 4096], F16, tag="P", name="usq")
                    nc.vector.tensor_tensor(out=usq[:], in0=up[:], in1=up[:],
                                            op=OP.mult)
                    nq = sm.tile([128, 256], F16, tag="nq", name="nq")
                    dtree2(HM5(usq), nq)
                    nc.vector.tensor_scalar_max(out=nq[:], in0=nq[:],
                                                scalar1=1e-8)
                    nc.vector.reciprocal(nq[:], nq[:])
                    nc.vector.tensor_tensor(out=rr[:], in0=dot[:], in1=nq[:],
                                            op=OP.mult)

                # softmax over i (shift-free: |rr| <= 8)
                es = sm.tile([128, 256], F16, tag="es", name="es")
                nc.scalar.activation(es[:], rr[:], AF.Exp)
                es4 = es.rearrange("p (k i n g) -> p k i n g", k=4, i=8, n=2)
                zt1 = trees.tile([128, 128], F16, tag="z4", name="z4")
                z14 = zt1.rearrange("p (k i n g) -> p k i n g", k=4, i=4, n=2)
                nc.vector.tensor_tensor(out=z14, in0=es4[:, :, 0:4],
                                        in1=es4[:, :, 4:8], op=OP.add)
                zt2 = trees.tile([128, 64], F16, tag="z2", name="z2")
                z24 = zt2.rearrange("p (k i n g) -> p k i n g", k=4, i=2, n=2)
                nc.vector.tensor_tensor(out=z24, in0=z14[:, :, 0:2],
                                        in1=z14[:, :, 2:4], op=OP.add)
                Z = sm.tile([128, 32], F16, tag="Z", name="Z")
                Z4 = Z.rearrange("p (k n g) -> p k n g", k=4, n=2).unsqueeze(2)
                nc.vector.tensor_tensor(out=Z4, in0=z24[:, :, 0:1],
                                        in1=z24[:, :, 1:2], op=OP.add)
                nc.vector.reciprocal(Z[:], Z[:])
                sc = sm.tile([128, 256], F16, tag="sc", name="sc")
                Zb = (Z.rearrange("p (k n g) -> p k n g", k=4, n=2).unsqueeze(2)
                      .broadcast_to((128, 4, 8, 2, 4)))
                sc4 = sc.rearrange("p (k i n g) -> p k i n g", k=4, i=8, n=2)
                nc.vector.tensor_tensor(out=sc4, in0=es4, in1=Zb, op=OP.mult)

                # s = sum_i score_i up_i ; squash over g
                P2 = scr.tile([128, 4096], F16, tag="P", name="P2")
                split_tt(HM5(P2), HM5(up), bc_kig2(sc), OP.mult, cut_b)
                s = sm.tile([128, 512], F16, tag="s", name="s")
                itree2(kin3(P2), s)
                ssq = sm.tile([128, 512], F16, tag="ssq", name="ssq")
                nc.scalar.activation(ssq[:], s[:], AF.Square)
                s4 = ssq.rearrange("p (knd g) -> p knd g", knd=128)
                gt = trees.tile([128, 256], F16, tag="g2", name="g2")
                gt4 = gt.rearrange("p (knd g) -> p knd g", knd=128)
                nc.vector.tensor_tensor(out=gt4, in0=s4[:, :, 0:2],
                                        in1=s4[:, :, 2:4], op=OP.add)
                nsq = sm.tile([128, 128], F16, tag="nsq", name="nsq")
                nsq4 = nsq[:].unsqueeze(2)
                nc.vector.tensor_tensor(out=nsq4, in0=gt4[:, :, 0:1],
                                        in1=gt4[:, :, 1:2], op=OP.add)
                sr = sm.tile([128, 128], F16, tag="sr", name="sr")
                nc.scalar.activation(sr[:], nsq[:], AF.Sqrt, bias=eps16[:])
                d1 = sm.tile([128, 128], F16, tag="d1", name="d1")
                nc.vector.scalar_tensor_tensor(
                    out=d1[:], in0=nsq[:], scalar=1.0, in1=sr[:],
                    op0=OP.add, op1=OP.mult)
                nc.vector.reciprocal(d1[:], d1[:])
                f = sm.tile([128, 128], F16, tag="f", name="f")
                nc.vector.tensor_tensor(out=f[:], in0=nsq[:], in1=d1[:], op=OP.mult)
                v = vp.tile([128, 512], F16, tag="v", name="v")
                v4 = v.rearrange("p (knd g) -> p knd g", knd=128)
                fb = f[:].unsqueeze(2).broadcast_to((128, 128, 4))
                s44 = s.rearrange("p (knd g) -> p knd g", knd=128)
                nc.vector.tensor_tensor(out=v4, in0=s44, in1=fb, op=OP.mult)
                u_ = bl * 1024 + h * 512
                nc.sync.dma_start(out=outd.ap()[:, u_:u_ + 512], in_=v[:])

            for bl in range(2):
                for chh in range(2):
                    conv_pass(bl, chh)
                    transS_half(bl, chh)
                    transT_half(bl, chh)
            route_half(0, 0, cut_a=32, cut_b=32)
            route_half(0, 1, cut_a=32, cut_b=24)
            route_half(1, 0, cut_a=24, cut_b=24)
            route_half(1, 1, cut_a=24, cut_b=24)

    _split_sync_waits(nc)
    return nc


def _pack_weights(conv_w):
    w = np.asarray(conv_w, np.float32)
    wt = np.stack(
        [np.roll(np.rot90(w, k=r, axes=(3, 4)), r, axis=2) for r in range(4)], axis=1
    )
    W512 = np.ascontiguousarray(wt.reshape(512, 64, 3, 3), dtype=np.float32)
    packs = []
    for pi in range(4):
        Wc = W512[128 * pi: 128 * pi + 128]  # 2 nouts' channels (n,d,g)
        w_pack = np.zeros((128, 6, 128), np.float32)
        for kx in range(3):
            w_pack[0:64, kx] = Wc[:, :, 0, kx].T
            w_pack[64:128, kx] = Wc[:, :, 1, kx].T
            w_pack[0:64, 3 + kx] = Wc[:, :, 2, kx].T
        packs.append(np.ascontiguousarray(
            w_pack.reshape(128, 768), dtype=np.float16))
    return packs


def _pack_caps(capsules):
    """[4,8,16,4,32,32] f32 -> [32,128,1164] f16 (padded image + row-shifted
    copy per [128]-partition tile)."""
    x = np.asarray(capsules, np.float32).reshape(32, 64, 32, 32)
    pad = np.zeros((32, 64, 34, 34), np.float16)
    pad[:, :, 1:33, 1:33] = x.astype(np.float16)
    A = pad.reshape(32, 64, 1156)
    buf = np.zeros((32, 128, 1164), np.float16)
    buf[:, 0:64, 0:1156] = A
    buf[:, 64:128, 0:1122] = A[:, :, 34:1156]
    return buf


_CACHE = {}


def kernel(capsules, conv_w, conv_b, ln_gamma, ln_beta):
    conv_b = np.asarray(conv_b, np.float32)
    ln_gamma = np.asarray(ln_gamma, np.float32)
    ln_beta = np.asarray(ln_beta, np.float32)
    apply_bias = bool(np.any(conv_b))
    apply_gb = bool(np.any(ln_gamma != 1.0) or np.any(ln_beta != 0.0))

    key = (apply_bias, apply_gb)
    if key not in _CACHE:
        _CACHE[key] = build_program(apply_bias=apply_bias, apply_gb=apply_gb)
    nc = _CACHE[key]

    capsd = _pack_caps(capsules)
    packs = _pack_weights(conv_w)
    identity = np.eye(128, dtype=np.float16)
    mmu = np.zeros((128, 16), np.float16)
    for ch in range(128):
        nn_, gg = ch // 64, ch % 4
        for t in range(2):
            mmu[ch, t * 8 + nn_ * 4 + gg] = 1.0 / 16.0
    in_maps = []
    for c in range(8):
        beta_ = c // 4   # batch-pair
        pi = c % 4       # nout-pair
        m = {"capsd": np.ascontiguousarray(capsd[16 * beta_: 16 * beta_ + 16]),
             "w": packs[pi], "ident": identity, "mmu": mmu}
        if apply_bias:
            # channel order (n,d,g): n*64 + d*4 + g
            b_loc = np.repeat(conv_b[32 * pi: 32 * pi + 32], 4).astype(np.float16)
            m["cb"] = np.ascontiguousarray(b_loc.reshape(1, 128))
        if apply_gb:
            m["gam"] = np.ascontiguousarray(ln_gamma.reshape(1, 16), dtype=np.float16)
            m["bet"] = np.ascontiguousarray(ln_beta.reshape(1, 16), dtype=np.float16)
        in_maps.append(m)

    res = run_bass_kernel_spmd(nc, in_maps, core_ids=list(range(8)), trace=False)
    # per-core out: [128, 2048] f16 = (p, bl, n, k, d, g); position = k*128+p
    out = np.zeros((4, 8, 16, 4, 32, 32), np.float32)
    for c in range(8):
        beta_, pi = c // 4, c % 4
        r = np.asarray(res.results[c]["outd"], np.float32).reshape(128, 2, 8, 2, 16, 4)
        for bl in range(2):
            for n in range(2):
                out[2 * beta_ + bl, 2 * pi + n] = (
                    r[:, bl, :, n].transpose(2, 3, 1, 0).reshape(16, 4, 32, 32))
    return out


# revision 40
# speedup vs baseline: 1.0370x; 1.0082x over previous
"""Trainium2 Bass kernel for nn_ConvolutionalCapsules.

Sharding: core c (of 8) owns output-capsules {2*(c%4), 2*(c%4)+1} for batches
{2*(c//4), 2*(c//4)+1}. Each core runs the p4 group conv with 128 output
channels (2 nout x 16 dout x 4 rot) over its 16 images (2 batches x 8 input
capsules), then LayerNorm + degree-score routing + squash per (batch, nout).

Conv: 3x3 conv as shifted matmuls from a zero-padded fp16 SBUF image (34x34
rows, dual-copy: partitions 0-63 = padded image, 64-127 = same shifted one row
so one K=128 matmul covers two filter taps). Weights are the stationary
operand ([K, 128 out-channels]), so each of the 6 matmuls per 512 positions
retires 2x the baseline's work. PE then transposes u ([128 chan, pos]) to the
routing layout ([128 pos, chan]) in fp16 (4x cheaper than f32 transposes).

Routing runs fully in fp16 on the DVE (tensor_tensor at the 2x perf mode,
tensor_scalar at 4x); segmented reductions over d/i/g are binary tree-adds of
strided views, which beat TensorReduce ~4x. LayerNorm is algebraically folded:
up = (T - mu)*rstd, rr_i = (up_i . S) * (var+eps)/(16 var), softmax over i
(shift-free: |rr| <= 8), s = sum_i score_i up_i, squash over g.

Host packs inputs (pad + dual-copy + fp16) and unpacks the [pos, (b,n,k,d,g)]
fp16 output, so every DMA moves contiguous >=512B lines.
"""

import numpy as np
from contextlib import ExitStack

import concourse.bass as bass
import concourse.tile as tile
from concourse import mybir
from concourse.bass_utils import run_bass_kernel_spmd

F16 = mybir.dt.float16
F32 = mybir.dt.float32
AF = mybir.ActivationFunctionType
OP = mybir.AluOpType

_ENGINES = {
    mybir.EngineType.PE,
    mybir.EngineType.Activation,
    mybir.EngineType.Pool,
    mybir.EngineType.DVE,
    mybir.EngineType.SP,
}


def _split_sync_waits(nc):
    """This walrus build accepts a single embedded sync-wait per instruction;
    hoist extras onto preceding NoOps on the same engine (ge-imm waits commute)."""
    for f in nc.m.functions:
        for bb in f.blocks:
            newl = []
            changed = False
            for inst in list(bb.instructions):
                si = inst.sync_info
                waits = list(si.on_wait) if si and si.on_wait else []
                if len(waits) > 1 and inst.engine in _ENGINES:
                    changed = True
                    for k, w in enumerate(waits[:-1]):
                        newl.append(
                            mybir.InstNoOp(
                                name=f"{inst.name}-ws{k}",
                                ins=[],
                                outs=[],
                                engine=inst.engine,
                                sync_info=mybir.SyncInfo(on_wait=[w], on_update=[]),
                            )
                        )
                    si.on_wait = waits[-1:]
                    inst.sync_info = si
                newl.append(inst)
            if changed:
                bb.instructions = newl


def build_program(apply_bias=False, apply_gb=False):
    nc = bass.Bass(trn_type="TRN2")
    capsd = nc.dram_tensor("capsd", [16, 128, 1164], F16, kind="ExternalInput")
    w = nc.dram_tensor("w", [128, 768], F16, kind="ExternalInput")
    ident = nc.dram_tensor("ident", [128, 128], F16, kind="ExternalInput")
    mmu = nc.dram_tensor("mmu", [128, 16], F16, kind="ExternalInput")
    if apply_bias:
        cb = nc.dram_tensor("cb", [1, 128], F16, kind="ExternalInput")
    if apply_gb:
        gam = nc.dram_tensor("gam", [1, 16], F16, kind="ExternalInput")
        bet = nc.dram_tensor("bet", [1, 16], F16, kind="ExternalInput")
    outd = nc.dram_tensor("outd", [128, 2048], F16, kind="ExternalOutput")

    with tile.TileContext(nc) as tc:
        with nc.allow_low_precision(reason="fp16 routing; 2e-2 rel tolerance"), \
             ExitStack() as ctx:
            consts = ctx.enter_context(tc.tile_pool(name="consts", bufs=1))
            imgs = ctx.enter_context(tc.tile_pool(name="imgs", bufs=1))
            us = ctx.enter_context(tc.tile_pool(name="us", bufs=1))
            ps = ctx.enter_context(tc.tile_pool(name="ps", bufs=4, space="PSUM"))
            tps = ctx.enter_context(tc.tile_pool(name="tps", bufs=3, space="PSUM"))
            sps = ctx.enter_context(tc.tile_pool(name="sps", bufs=1, space="PSUM"))
            Tp = ctx.enter_context(tc.tile_pool(name="Tp", bufs=1))
            qp = ctx.enter_context(tc.tile_pool(name="qp", bufs=2))
            scr = ctx.enter_context(tc.tile_pool(name="scr", bufs=3))
            trees = ctx.enter_context(tc.tile_pool(name="trees", bufs=2))
            sm = ctx.enter_context(tc.tile_pool(name="sm", bufs=3))
            vp = ctx.enter_context(tc.tile_pool(name="vp", bufs=2))

            w_sb = consts.tile([128, 768], F16, tag="w")
            nc.sync.dma_start(out=w_sb[:], in_=w.ap())
            id_sb = consts.tile([128, 128], F16, tag="ident")
            nc.sync.dma_start(out=id_sb[:], in_=ident.ap())
            mmu_sb = consts.tile([128, 16], F16, tag="mmu")
            nc.sync.dma_start(out=mmu_sb[:], in_=mmu.ap())
            eps5 = consts.tile([128, 1], F32, tag="eps5")
            nc.vector.memset(eps5[:], 1e-5)
            eps16 = consts.tile([128, 1], F32, tag="eps16")
            nc.vector.memset(eps16[:], 1e-16)
            if apply_bias:
                cb_sb = consts.tile([1, 128], F16, tag="cb")
                nc.sync.dma_start(out=cb_sb[:], in_=cb.ap())
                ones512 = consts.tile([1, 512], F16, tag="ones512")
                nc.vector.memset(ones512[:], 1.0)
            if apply_gb:
                gam_sb = consts.tile([128, 16], F16, tag="gam")
                nc.sync.dma_start(out=gam_sb[:], in_=gam.ap().partition_broadcast(128))
                bet_sb = consts.tile([128, 16], F16, tag="bet")
                nc.sync.dma_start(out=bet_sb[:], in_=bet.ap().partition_broadcast(128))

            def hview(ap_flat, o, rows):
                """[P, rows, 32] window at flat offset o, padded row stride 34."""
                return ap_flat[:, o: o + rows * 34].rearrange(
                    "c (h w) -> c h w", w=34
                )[:, :, 0:32]

            u_tiles = {}
            usq_tiles = {}
            T_tiles = {}
            stat_tiles = {}

            x_tiles = {}

            def conv_pass(bl, chh):
                """One 512-position chunk (4 k-blocks) of the conv for all 8
                images of batch bl. chh=0 covers k 0-3, chh=1 covers k 4-7."""
                base = chh * 16 * 34
                for i in range(8):
                    m = bl * 8 + i
                    if chh == 0:
                        xi = imgs.tile([128, 1164], F16, tag=f"x{i}", name=f"x{i}")
                        nc.sync.dma_start(out=xi[:], in_=capsd.ap()[m])
                        x_tiles[m] = xi
                        u = us.tile([128, 1024], F16, tag=f"u{i}", name=f"u{i}")
                        u_tiles[m] = u
                        usq = us.tile([128, 1024], F16, tag=f"usq{i}",
                                      name=f"usq{i}")
                        usq_tiles[m] = usq
                    xi, u, usq = x_tiles[m], u_tiles[m], usq_tiles[m]
                    p = ps.tile([128, 512], F32, tag="ps", name="ps")
                    for kx in range(3):
                        nc.tensor.matmul(
                            p[:],
                            lhsT=w_sb[:, kx * 128:(kx + 1) * 128],
                            rhs=hview(xi, base + kx, 16),
                            start=(kx == 0), stop=False)
                    for kx in range(3):
                        last = (kx == 2) and not apply_bias
                        nc.tensor.matmul(
                            p[:],
                            lhsT=w_sb[0:64, (3 + kx) * 128:(4 + kx) * 128],
                            rhs=hview(xi[0:64], base + 68 + kx, 16),
                            start=False, stop=last)
                    if apply_bias:
                        nc.tensor.matmul(
                            p[:], lhsT=cb_sb[:], rhs=ones512[:],
                            start=False, stop=True)
                    lo = chh * 512
                    nc.scalar.activation(u[:, lo:lo + 512], p[:], AF.Copy)
                    nc.gpsimd.tensor_tensor(
                        out=usq[:, lo:lo + 512], in0=u[:, lo:lo + 512],
                        in1=u[:, lo:lo + 512], op=OP.mult)

            def transS_half(bl, h):
                """LN stats (mean, mean-square over d) for k-blocks h*4..h*4+3
                on the PE via Mmu matmuls against u and u^2."""
                if h == 0:
                    stats = Tp.tile([128, 1024], F16, tag=f"st{bl}",
                                    name=f"st{bl}")
                    stat_tiles[bl] = stats
                stats = stat_tiles[bl]
                sp_ = sps.tile([128, 512], F32, tag="sps", name="sps")
                for kk in range(4):
                    k = h * 4 + kk
                    for j in range(8):
                        m = bl * 8 + j
                        co = kk * 128 + j * 16
                        nc.tensor.matmul(
                            sp_[:, co: co + 8],
                            lhsT=u_tiles[m][:, k * 128:(k + 1) * 128],
                            rhs=mmu_sb[:, 0:8], start=True, stop=True)
                        nc.tensor.matmul(
                            sp_[:, co + 8: co + 16],
                            lhsT=usq_tiles[m][:, k * 128:(k + 1) * 128],
                            rhs=mmu_sb[:, 8:16], start=True, stop=True)
                nc.scalar.activation(
                    stats[:, h * 512:(h + 1) * 512], sp_[:], AF.Copy)

            def transT_half(bl, h):
                """Transpose k-blocks h*4..h*4+3 of u into T[pos, (i,n,d,g)]."""
                if h == 0:
                    T = Tp.tile([128, 8192], F16, tag=f"T{bl}", name=f"T{bl}")
                    T_tiles[bl] = T
                T = T_tiles[bl]
                for kk in range(4):
                    k = h * 4 + kk
                    tp_ = tps.tile([128, 1024], F16, tag="tps", name="tps")
                    for j in range(8):
                        m = bl * 8 + j
                        nc.tensor.transpose(
                            out=tp_[:, j * 128:(j + 1) * 128],
                            in_=u_tiles[m][:, k * 128:(k + 1) * 128],
                            identity=id_sb[:])
                    nc.scalar.activation(
                        T[:, k * 1024:(k + 1) * 1024], tp_[:], AF.Copy)

            # ---- half-pair routing: both nout units of a batch for 4 of the
            # 8 position-blocks, as one set of double-width ops. Four such
            # chains pipeline against each other and the conv phases. ----

            def kig2(t):  # [128,256] cols (ki32, n, g)
                return t.rearrange("p (ki n g) -> p ki n g", ki=32, n=2)

            def bc_kig2(t):  # [128,256] (ki32,n,g) -> [p,ki,n,d,g]
                return (kig2(t).unsqueeze(3)
                        .broadcast_to((128, 32, 2, 16, 4)))

            def TM5(t):  # [128,8192] cols (k,i,n,d,g) -> [p,ki64,n,d,g]
                return t.rearrange("p (ki n d g) -> p ki n d g", ki=64, n=2, d=16)

            def HM5(t):  # [128,4096] half tile -> [p,ki32,n,d,g]
                return t.rearrange("p (ki n d g) -> p ki n d g", ki=32, n=2, d=16)

            def kin3(t):  # [128,4096] half tile -> [p,k4,i,(ndg)]
                return t.rearrange("p (k i ndg) -> p k i ndg", k=4, i=8)

            def itree2(src3, out_kndg):
                """Sum over i of [p,k4,i,(ndg)] via tree-adds."""
                t1 = trees.tile([128, 2048], F16, tag="t2048", name="t2048")
                t13 = t1.rearrange("p (k i ndg) -> p k i ndg", k=4, i=4)
                nc.vector.tensor_tensor(out=t13, in0=src3[:, :, 0:4, :],
                                        in1=src3[:, :, 4:8, :], op=OP.add)
                t2 = trees.tile([128, 1024], F16, tag="t1024", name="t1024")
                t23 = t2.rearrange("p (k i ndg) -> p k i ndg", k=4, i=2)
                nc.vector.tensor_tensor(out=t23, in0=t13[:, :, 0:2, :],
                                        in1=t13[:, :, 2:4, :], op=OP.add)
                o3 = out_kndg.rearrange("p (k ndg) -> p k ndg", k=4).unsqueeze(2)
                nc.vector.tensor_tensor(out=o3, in0=t23[:, :, 0:1, :],
                                        in1=t23[:, :, 1:2, :], op=OP.add)

            def dtree2(src5, out_king):
                """Sum over d of [p,ki32,n,d,g] via tree-adds."""
                cur = src5
                nd = 16
                szs = {8: "t2048", 4: "t1024", 2: "t512"}
                while nd > 2:
                    nd //= 2
                    t = trees.tile([128, 32 * 2 * nd * 4], F16, tag=szs[nd],
                                   name=szs[nd])
                    t5 = t.rearrange("p (ki n d g) -> p ki n d g", ki=32, n=2,
                                     d=nd)
                    nc.vector.tensor_tensor(
                        out=t5, in0=cur[:, :, :, 0:nd, :],
                        in1=cur[:, :, :, nd:2 * nd, :], op=OP.add)
                    cur = t5
                o5 = kig2(out_king).unsqueeze(3)
                nc.vector.tensor_tensor(
                    out=o5, in0=cur[:, :, :, 0:1, :], in1=cur[:, :, :, 1:2, :],
                    op=OP.add)

            def split_tt(out5, in05, in15, op, cut):
                """One [p,ki32,n,d,g] TT, split: ki<cut on DVE, rest on Pool
                (concurrent engines, no serial coupling)."""
                if cut >= 32:
                    nc.vector.tensor_tensor(out=out5, in0=in05, in1=in15, op=op)
                    return
                nc.vector.tensor_tensor(
                    out=out5[:, 0:cut], in0=in05[:, 0:cut], in1=in15[:, 0:cut],
                    op=op)
                nc.gpsimd.tensor_tensor(
                    out=out5[:, cut:32], in0=in05[:, cut:32],
                    in1=in15[:, cut:32], op=op)

            def route_half(bl, h, cut_a, cut_b):
                T5 = TM5(T_tiles[bl])[:, h * 32:(h + 1) * 32]
                stM = stat_tiles[bl].rearrange(
                    "p (ki t n g) -> p ki t n g", ki=64, t=2, n=2)
                m1v = stM[:, h * 32:(h + 1) * 32, 0]   # [p,ki32,n,g]
                e2v = stM[:, h * 32:(h + 1) * 32, 1]

                # LayerNorm: var = E[t^2] - m1^2 ; rstd ; n2 = m1*rstd
                mm_ = sm.tile([128, 256], F16, tag="mm_", name="mm_")
                nc.vector.tensor_tensor(out=kig2(mm_), in0=m1v, in1=m1v,
                                        op=OP.mult)
                var = sm.tile([128, 256], F16, tag="var", name="var")
                nc.vector.tensor_tensor(out=kig2(var), in0=e2v, in1=kig2(mm_),
                                        op=OP.subtract)
                rstd = sm.tile([128, 256], F16, tag="rstd", name="rstd")
                nc.scalar.activation(rstd[:], var[:], AF.Sqrt, bias=eps5[:])
                nc.vector.reciprocal(rstd[:], rstd[:])
                n2 = sm.tile([128, 256], F16, tag="n2", name="n2")
                nc.vector.tensor_tensor(out=kig2(n2), in0=m1v, in1=kig2(rstd),
                                        op=OP.mult)

                # up = (T - m1) * rstd (optionally * gamma + beta)
                q = qp.tile([128, 4096], F16, tag="q", name="q")
                split_tt(HM5(q), T5, bc_kig2(rstd), OP.mult, cut_a)
                up = qp.tile([128, 4096], F16, tag="up", name="up")
                split_tt(HM5(up), HM5(q), bc_kig2(n2), OP.subtract, cut_a)
                if apply_gb:
                    up5 = HM5(up)
                    gb = (gam_sb[:].unsqueeze(1).unsqueeze(2).unsqueeze(4)
                          .broadcast_to((128, 32, 2, 16, 4)))
                    bb_ = (bet_sb[:].unsqueeze(1).unsqueeze(2).unsqueeze(4)
                           .broadcast_to((128, 32, 2, 16, 4)))
                    nc.vector.tensor_tensor(out=up5, in0=up5, in1=gb, op=OP.mult)
                    nc.vector.tensor_tensor(out=up5, in0=up5, in1=bb_, op=OP.add)

                # S = sum_i up ; dot_i = up_i . S
                S = sm.tile([128, 512], F16, tag="S", name="S")
                itree2(kin3(up), S)
                P = scr.tile([128, 4096], F16, tag="P", name="P")
                S_bc = (S.rearrange("p (k ndg) -> p k ndg", k=4).unsqueeze(2)
                        .broadcast_to((128, 4, 8, 128)))
                kc = cut_b // 8
                if kc >= 4:
                    nc.vector.tensor_tensor(out=kin3(P), in0=kin3(up),
                                            in1=S_bc, op=OP.mult)
                else:
                    nc.vector.tensor_tensor(
                        out=kin3(P)[:, 0:kc], in0=kin3(up)[:, 0:kc],
                        in1=S_bc[:, 0:kc], op=OP.mult)
                    nc.gpsimd.tensor_tensor(
                        out=kin3(P)[:, kc:4], in0=kin3(up)[:, kc:4],
                        in1=S_bc[:, kc:4], op=OP.mult)
                dot = sm.tile([128, 256], F16, tag="dot", name="dot")
                dtree2(HM5(P), dot)

                # rr_i = dot_i / max(||up_i||^2, 1e-8)
                rr = sm.tile([128, 256], F16, tag="rr", name="rr")
                if not apply_gb:
                    # 1/||up||^2 = (var+eps)/(16 var) = 1/16 + (eps/16)/var
                    ns_ = sm.tile([128, 256], F16, tag="ns_", name="ns_")
                    nc.vector.reciprocal(ns_[:], var[:])
                    nc.vector.tensor_scalar(
                        out=ns_[:], in0=ns_[:], scalar1=1e-5 / 16.0,
                        scalar2=1.0 / 16.0, op0=OP.mult, op1=OP.add)
                    nc.vector.tensor_tensor(out=rr[:], in0=dot[:], in1=ns_[:],
                                            op=OP.mult)
                else:
                    usq = scr.tile([128, 4096], F16, tag="P", name="usq")
                    nc.vector.tensor_tensor(out=usq[:], in0=up[:], in1=up[:],
                                            op=OP.mult)
                    nq = sm.tile([128, 256], F16, tag="nq", name="nq")
                    dtree2(HM5(usq), nq)
                    nc.vector.tensor_scalar_max(out=nq[:], in0=nq[:],
                                                scalar1=1e-8)
                    nc.vector.reciprocal(nq[:], nq[:])
                    nc.vector.tensor_tensor(out=rr[:], in0=dot[:], in1=nq[:],
                                            op=OP.mult)

                # softmax over i (shift-free: |rr| <= 8)
                es = sm.tile([128, 256], F16, tag="es", name="es")
                nc.scalar.activation(es[:], rr[:], AF.Exp)
                es4 = es.rearrange("p (k i n g) -> p k i n g", k=4, i=8, n=2)
                zt1 = trees.tile([128, 128], F16, tag="z4", name="z4")
                z14 = zt1.rearrange("p (k i n g) -> p k i n g", k=4, i=4, n=2)
                nc.vector.tensor_tensor(out=z14, in0=es4[:, :, 0:4],
                                        in1=es4[:, :, 4:8], op=OP.add)
                zt2 = trees.tile([128, 64], F16, tag="z2", name="z2")
                z24 = zt2.rearrange("p (k i n g) -> p k i n g", k=4, i=2, n=2)
                nc.vector.tensor_tensor(out=z24, in0=z14[:, :, 0:2],
                                        in1=z14[:, :, 2:4], op=OP.add)
                Z = sm.tile([128, 32], F16, tag="Z", name="Z")
                Z4 = Z.rearrange("p (k n g) -> p k n g", k=4, n=2).unsqueeze(2)
                nc.vector.tensor_tensor(out=Z4, in0=z24[:, :, 0:1],
                                        in1=z24[:, :, 1:2], op=OP.add)
                nc.vector.reciprocal(Z[:], Z[:])
                sc = sm.tile([128, 256], F16, tag="sc", name="sc")
                Zb = (Z.rearrange("p (k n g) -> p k n g", k=4, n=2).unsqueeze(2)
                      .broadcast_to((128, 4, 8, 2, 4)))
                sc4 = sc.rearrange("p (k i n g) -> p k i n g", k=4, i=8, n=2)
                nc.vector.tensor_tensor(out=sc4, in0=es4, in1=Zb, op=OP.mult)

                # s = sum_i score_i up_i ; squash over g
                P2 = scr.tile([128, 4096], F16, tag="P", name="P2")
                split_tt(HM5(P2), HM5(up), bc_kig2(sc), OP.mult, cut_b)
                s = sm.tile([128, 512], F16, tag="s", name="s")
                itree2(kin3(P2), s)
                ssq = sm.tile([128, 512], F16, tag="ssq", name="ssq")
                nc.scalar.activation(ssq[:], s[:], AF.Square)
                s4 = ssq.rearrange("p (knd g) -> p knd g", knd=128)
                gt = trees.tile([128, 256], F16, tag="g2", name="g2")
                gt4 = gt.rearrange("p (knd g) -> p knd g", knd=128)
                nc.vector.tensor_tensor(out=gt4, in0=s4[:, :, 0:2],
                                        in1=s4[:, :, 2:4], op=OP.add)
                nsq = sm.tile([128, 128], F16, tag="nsq", name="nsq")
                nsq4 = nsq[:].unsqueeze(2)
                nc.vector.tensor_tensor(out=nsq4, in0=gt4[:, :, 0:1],
                                        in1=gt4[:, :, 1:2], op=OP.add)
                sr = sm.tile([128, 128], F16, tag="sr", name="sr")
                nc.scalar.activation(sr[:], nsq[:], AF.Sqrt, bias=eps16[:])
                d1 = sm.tile([128, 128], F16, tag="d1", name="d1")
                nc.vector.scalar_tensor_tensor(
                    out=d1[:], in0=nsq[:], scalar=1.0, in1=sr[:],
                    op0=OP.add, op1=OP.mult)
                nc.vector.reciprocal(d1[:], d1[:])
                f = sm.tile([128, 128], F16, tag="f", name="f")
                nc.vector.tensor_tensor(out=f[:], in0=nsq[:], in1=d1[:], op=OP.mult)
                v = vp.tile([128, 512], F16, tag="v", name="v")
                v4 = v.rearrange("p (knd g) -> p knd g", knd=128)
                fb = f[:].unsqueeze(2).broadcast_to((128, 128, 4))
                s44 = s.rearrange("p (knd g) -> p knd g", knd=128)
                nc.vector.tensor_tensor(out=v4, in0=s44, in1=fb, op=OP.mult)
                u_ = bl * 1024 + h * 512
                nc.sync.dma_start(out=outd.ap()[:, u_:u_ + 512], in_=v[:])

            for bl in range(2):
                for chh in range(2):
                    conv_pass(bl, chh)
                    transS_half(bl, chh)
                    transT_half(bl, chh)
            route_half(0, 0, cut_a=32, cut_b=32)
            route_half(0, 1, cut_a=32, cut_b=24)
            route_half(1, 0, cut_a=24, cut_b=24)
            route_half(1, 1, cut_a=24, cut_b=24)

    _split_sync_waits(nc)
    return nc


def _pack_weights(conv_w):
    w = np.asarray(conv_w, np.float32)
    wt = np.stack(
        [np.roll(np.rot90(w, k=r, axes=(3, 4)), r, axis=2) for r in range(4)], axis=1
    )
    W512 = np.ascontiguousarray(wt.reshape(512, 64, 3, 3), dtype=np.float32)
    packs = []
    for pi in range(4):
        Wc = W512[128 * pi: 128 * pi + 128]  # 2 nouts' channels (n,d,g)
        w_pack = np.zeros((128, 6, 128), np.float32)
        for kx in range(3):
            w_pack[0:64, kx] = Wc[:, :, 0, kx].T
            w_pack[64:128, kx] = Wc[:, :, 1, kx].T
            w_pack[0:64, 3 + kx] = Wc[:, :, 2, kx].T
        packs.append(np.ascontiguousarray(
            w_pack.reshape(128, 768), dtype=np.float16))
    return packs


def _pack_caps(capsules):
    """[4,8,16,4,32,32] f32 -> [32,128,1164] f16 (padded image + row-shifted
    copy per [128]-partition tile)."""
    x = np.asarray(capsules, np.float32).reshape(32, 64, 32, 32)
    pad = np.zeros((32, 64, 34, 34), np.float16)
    pad[:, :, 1:33, 1:33] = x.astype(np.float16)
    A = pad.reshape(32, 64, 1156)
    buf = np.zeros((32, 128, 1164), np.float16)
    buf[:, 0:64, 0:1156] = A
    buf[:, 64:128, 0:1122] = A[:, :, 34:1156]
    return buf


_CACHE = {}


def kernel(capsules, conv_w, conv_b, ln_gamma, ln_beta):
    conv_b = np.asarray(conv_b, np.float32)
    ln_gamma = np.asarray(ln_gamma, np.float32)
    ln_beta = np.asarray(ln_beta, np.float32)
    apply_bias = bool(np.any(conv_b))
    apply_gb = bool(np.any(ln_gamma != 1.0) or np.any(ln_beta != 0.0))

    key = (apply_bias, apply_gb)
    if key not in _CACHE:
        _CACHE[key] = build_program(apply_bias=apply_bias, apply_gb=apply_gb)
    nc = _CACHE[key]

    capsd = _pack_caps(capsules)
    packs = _pack_weights(conv_w)
    identity = np.eye(128, dtype=np.float16)
    mmu = np.zeros((128, 16), np.float16)
    for ch in range(128):
        nn_, gg = ch // 64, ch % 4
        for t in range(2):
            mmu[ch, t * 8 + nn_ * 4 + gg] = 1.0 / 16.0
    in_maps = []
    for c in range(8):
        beta_ = c // 4   # batch-pair
        pi = c % 4       # nout-pair
        m = {"capsd": np.ascontiguousarray(capsd[16 * beta_: 16 * beta_ + 16]),
             "w": packs[pi], "ident": identity, "mmu": mmu}
        if apply_bias:
            # channel order (n,d,g): n*64 + d*4 + g
            b_loc = np.repeat(conv_b[32 * pi: 32 * pi + 32], 4).astype(np.float16)
            m["cb"] = np.ascontiguousarray(b_loc.reshape(1, 128))
        if apply_gb:
            m["gam"] = np.ascontiguousarray(ln_gamma.reshape(1, 16), dtype=np.float16)
            m["bet"] = np.ascontiguousarray(ln_beta.reshape(1, 16), dtype=np.float16)
        in_maps.append(m)

    res = run_bass_kernel_spmd(nc, in_maps, core_ids=list(range(8)), trace=False)
    # per-core out: [128, 2048] f16 = (p, bl, n, k, d, g); position = k*128+p
    out = np.zeros((4, 8, 16, 4, 32, 32), np.float32)
    for c in range(8):
        beta_, pi = c // 4, c % 4
        r = np.asarray(res.results[c]["outd"], np.float32).reshape(128, 2, 8, 2, 16, 4)
        for bl in range(2):
            for n in range(2):
                out[2 * beta_ + bl, 2 * pi + n] = (
                    r[:, bl, :, n].transpose(2, 3, 1, 0).reshape(16, 4, 32, 32))
    return out


# revision 41
# speedup vs baseline: 1.0385x; 1.0015x over previous
"""Trainium2 Bass kernel for nn_ConvolutionalCapsules.

Sharding: core c (of 8) owns output-capsules {2*(c%4), 2*(c%4)+1} for batches
{2*(c//4), 2*(c//4)+1}. Each core runs the p4 group conv with 128 output
channels (2 nout x 16 dout x 4 rot) over its 16 images (2 batches x 8 input
capsules), then LayerNorm + degree-score routing + squash per (batch, nout).

Conv: 3x3 conv as shifted matmuls from a zero-padded fp16 SBUF image (34x34
rows, dual-copy: partitions 0-63 = padded image, 64-127 = same shifted one row
so one K=128 matmul covers two filter taps). Weights are the stationary
operand ([K, 128 out-channels]), so each of the 6 matmuls per 512 positions
retires 2x the baseline's work. PE then transposes u ([128 chan, pos]) to the
routing layout ([128 pos, chan]) in fp16 (4x cheaper than f32 transposes).

Routing runs fully in fp16 on the DVE (tensor_tensor at the 2x perf mode,
tensor_scalar at 4x); segmented reductions over d/i/g are binary tree-adds of
strided views, which beat TensorReduce ~4x. LayerNorm is algebraically folded:
up = (T - mu)*rstd, rr_i = (up_i . S) * (var+eps)/(16 var), softmax over i
(shift-free: |rr| <= 8), s = sum_i score_i up_i, squash over g.

Host packs inputs (pad + dual-copy + fp16) and unpacks the [pos, (b,n,k,d,g)]
fp16 output, so every DMA moves contiguous >=512B lines.
"""

import numpy as np
from contextlib import ExitStack

import concourse.bass as bass
import concourse.tile as tile
from concourse import mybir
from concourse.bass_utils import run_bass_kernel_spmd

F16 = mybir.dt.float16
F32 = mybir.dt.float32
AF = mybir.ActivationFunctionType
OP = mybir.AluOpType

_ENGINES = {
    mybir.EngineType.PE,
    mybir.EngineType.Activation,
    mybir.EngineType.Pool,
    mybir.EngineType.DVE,
    mybir.EngineType.SP,
}


def _split_sync_waits(nc):
    """This walrus build accepts a single embedded sync-wait per instruction;
    hoist extras onto preceding NoOps on the same engine (ge-imm waits commute)."""
    for f in nc.m.functions:
        for bb in f.blocks:
            newl = []
            changed = False
            for inst in list(bb.instructions):
                si = inst.sync_info
                waits = list(si.on_wait) if si and si.on_wait else []
                if len(waits) > 1 and inst.engine in _ENGINES:
                    changed = True
                    for k, w in enumerate(waits[:-1]):
                        newl.append(
                            mybir.InstNoOp(
                                name=f"{inst.name}-ws{k}",
                                ins=[],
                                outs=[],
                                engine=inst.engine,
                                sync_info=mybir.SyncInfo(on_wait=[w], on_update=[]),
                            )
                        )
                    si.on_wait = waits[-1:]
                    inst.sync_info = si
                newl.append(inst)
            if changed:
                bb.instructions = newl


def build_program(apply_bias=False, apply_gb=False):
    nc = bass.Bass(trn_type="TRN2")
    capsd = nc.dram_tensor("capsd", [16, 128, 1164], F16, kind="ExternalInput")
    w = nc.dram_tensor("w", [128, 768], F16, kind="ExternalInput")
    ident = nc.dram_tensor("ident", [128, 128], F16, kind="ExternalInput")
    mmu = nc.dram_tensor("mmu", [128, 16], F16, kind="ExternalInput")
    if apply_bias:
        cb = nc.dram_tensor("cb", [1, 128], F16, kind="ExternalInput")
    if apply_gb:
        gam = nc.dram_tensor("gam", [1, 16], F16, kind="ExternalInput")
        bet = nc.dram_tensor("bet", [1, 16], F16, kind="ExternalInput")
    outd = nc.dram_tensor("outd", [128, 2048], F16, kind="ExternalOutput")

    with tile.TileContext(nc) as tc:
        with nc.allow_low_precision(reason="fp16 routing; 2e-2 rel tolerance"), \
             ExitStack() as ctx:
            consts = ctx.enter_context(tc.tile_pool(name="consts", bufs=1))
            imgs = ctx.enter_context(tc.tile_pool(name="imgs", bufs=1))
            us = ctx.enter_context(tc.tile_pool(name="us", bufs=1))
            ps = ctx.enter_context(tc.tile_pool(name="ps", bufs=4, space="PSUM"))
            tps = ctx.enter_context(tc.tile_pool(name="tps", bufs=3, space="PSUM"))
            sps = ctx.enter_context(tc.tile_pool(name="sps", bufs=1, space="PSUM"))
            Tp = ctx.enter_context(tc.tile_pool(name="Tp", bufs=1))
            qp = ctx.enter_context(tc.tile_pool(name="qp", bufs=2))
            scr = ctx.enter_context(tc.tile_pool(name="scr", bufs=3))
            trees = ctx.enter_context(tc.tile_pool(name="trees", bufs=2))
            sm = ctx.enter_context(tc.tile_pool(name="sm", bufs=3))
            vp = ctx.enter_context(tc.tile_pool(name="vp", bufs=2))

            w_sb = consts.tile([128, 768], F16, tag="w")
            nc.sync.dma_start(out=w_sb[:], in_=w.ap())
            id_sb = consts.tile([128, 128], F16, tag="ident")
            nc.sync.dma_start(out=id_sb[:], in_=ident.ap())
            mmu_sb = consts.tile([128, 16], F16, tag="mmu")
            nc.sync.dma_start(out=mmu_sb[:], in_=mmu.ap())
            eps5 = consts.tile([128, 1], F32, tag="eps5")
            nc.vector.memset(eps5[:], 1e-5)
            eps16 = consts.tile([128, 1], F32, tag="eps16")
            nc.vector.memset(eps16[:], 1e-16)
            if apply_bias:
                cb_sb = consts.tile([1, 128], F16, tag="cb")
                nc.sync.dma_start(out=cb_sb[:], in_=cb.ap())
                ones512 = consts.tile([1, 512], F16, tag="ones512")
                nc.vector.memset(ones512[:], 1.0)
            if apply_gb:
                gam_sb = consts.tile([128, 16], F16, tag="gam")
                nc.sync.dma_start(out=gam_sb[:], in_=gam.ap().partition_broadcast(128))
                bet_sb = consts.tile([128, 16], F16, tag="bet")
                nc.sync.dma_start(out=bet_sb[:], in_=bet.ap().partition_broadcast(128))

            def hview(ap_flat, o, rows):
                """[P, rows, 32] window at flat offset o, padded row stride 34."""
                return ap_flat[:, o: o + rows * 34].rearrange(
                    "c (h w) -> c h w", w=34
                )[:, :, 0:32]

            u_tiles = {}
            usq_tiles = {}
            T_tiles = {}
            stat_tiles = {}

            x_tiles = {}

            def conv_pass(bl, chh):
                """One 512-position chunk (4 k-blocks) of the conv for all 8
                images of batch bl. chh=0 covers k 0-3, chh=1 covers k 4-7."""
                base = chh * 16 * 34
                for i in range(8):
                    m = bl * 8 + i
                    if chh == 0:
                        xi = imgs.tile([128, 1164], F16, tag=f"x{i}", name=f"x{i}")
                        nc.sync.dma_start(out=xi[:], in_=capsd.ap()[m])
                        x_tiles[m] = xi
                        u = us.tile([128, 1024], F16, tag=f"u{i}", name=f"u{i}")
                        u_tiles[m] = u
                        usq = us.tile([128, 1024], F16, tag=f"usq{i}",
                                      name=f"usq{i}")
                        usq_tiles[m] = usq
                    xi, u, usq = x_tiles[m], u_tiles[m], usq_tiles[m]
                    p = ps.tile([128, 512], F32, tag="ps", name="ps")
                    for kx in range(3):
                        nc.tensor.matmul(
                            p[:],
                            lhsT=w_sb[:, kx * 128:(kx + 1) * 128],
                            rhs=hview(xi, base + kx, 16),
                            start=(kx == 0), stop=False)
                    for kx in range(3):
                        last = (kx == 2) and not apply_bias
                        nc.tensor.matmul(
                            p[:],
                            lhsT=w_sb[0:64, (3 + kx) * 128:(4 + kx) * 128],
                            rhs=hview(xi[0:64], base + 68 + kx, 16),
                            start=False, stop=last)
                    if apply_bias:
                        nc.tensor.matmul(
                            p[:], lhsT=cb_sb[:], rhs=ones512[:],
                            start=False, stop=True)
                    lo = chh * 512
                    nc.scalar.activation(u[:, lo:lo + 512], p[:], AF.Copy)
                    nc.gpsimd.tensor_tensor(
                        out=usq[:, lo:lo + 512], in0=u[:, lo:lo + 512],
                        in1=u[:, lo:lo + 512], op=OP.mult)

            def transS_half(bl, h):
                """LN stats (mean, mean-square over d) for k-blocks h*4..h*4+3
                on the PE via Mmu matmuls against u and u^2."""
                if h == 0:
                    stats = Tp.tile([128, 1024], F16, tag=f"st{bl}",
                                    name=f"st{bl}")
                    stat_tiles[bl] = stats
                stats = stat_tiles[bl]
                sp_ = sps.tile([128, 512], F32, tag="sps", name="sps")
                for kk in range(4):
                    k = h * 4 + kk
                    for j in range(8):
                        m = bl * 8 + j
                        co = kk * 128 + j * 16
                        nc.tensor.matmul(
                            sp_[:, co: co + 8],
                            lhsT=u_tiles[m][:, k * 128:(k + 1) * 128],
                            rhs=mmu_sb[:, 0:8], start=True, stop=True)
                        nc.tensor.matmul(
                            sp_[:, co + 8: co + 16],
                            lhsT=usq_tiles[m][:, k * 128:(k + 1) * 128],
                            rhs=mmu_sb[:, 8:16], start=True, stop=True)
                nc.scalar.activation(
                    stats[:, h * 512:(h + 1) * 512], sp_[:], AF.Copy)

            def transT_half(bl, h):
                """Transpose k-blocks h*4..h*4+3 of u into T[pos, (i,n,d,g)]."""
                if h == 0:
                    T = Tp.tile([128, 8192], F16, tag=f"T{bl}", name=f"T{bl}")
                    T_tiles[bl] = T
                T = T_tiles[bl]
                for kk in range(4):
                    k = h * 4 + kk
                    tp_ = tps.tile([128, 1024], F16, tag="tps", name="tps")
                    for j in range(8):
                        m = bl * 8 + j
                        nc.tensor.transpose(
                            out=tp_[:, j * 128:(j + 1) * 128],
                            in_=u_tiles[m][:, k * 128:(k + 1) * 128],
                            identity=id_sb[:])
                    nc.scalar.activation(
                        T[:, k * 1024:(k + 1) * 1024], tp_[:], AF.Copy)

            # ---- half-pair routing: both nout units of a batch for 4 of the
            # 8 position-blocks, as one set of double-width ops. Four such
            # chains pipeline against each other and the conv phases. ----

            def kig2(t):  # [128,256] cols (ki32, n, g)
                return t.rearrange("p (ki n g) -> p ki n g", ki=32, n=2)

            def bc_kig2(t):  # [128,256] (ki32,n,g) -> [p,ki,n,d,g]
                return (kig2(t).unsqueeze(3)
                        .broadcast_to((128, 32, 2, 16, 4)))

            def TM5(t):  # [128,8192] cols (k,i,n,d,g) -> [p,ki64,n,d,g]
                return t.rearrange("p (ki n d g) -> p ki n d g", ki=64, n=2, d=16)

            def HM5(t):  # [128,4096] half tile -> [p,ki32,n,d,g]
                return t.rearrange("p (ki n d g) -> p ki n d g", ki=32, n=2, d=16)

            def kin3(t):  # [128,4096] half tile -> [p,k4,i,(ndg)]
                return t.rearrange("p (k i ndg) -> p k i ndg", k=4, i=8)

            def itree2(src3, out_kndg):
                """Sum over i of [p,k4,i,(ndg)] via tree-adds."""
                t1 = trees.tile([128, 2048], F16, tag="t2048", name="t2048")
                t13 = t1.rearrange("p (k i ndg) -> p k i ndg", k=4, i=4)
                nc.vector.tensor_tensor(out=t13, in0=src3[:, :, 0:4, :],
                                        in1=src3[:, :, 4:8, :], op=OP.add)
                t2 = trees.tile([128, 1024], F16, tag="t1024", name="t1024")
                t23 = t2.rearrange("p (k i ndg) -> p k i ndg", k=4, i=2)
                nc.vector.tensor_tensor(out=t23, in0=t13[:, :, 0:2, :],
                                        in1=t13[:, :, 2:4, :], op=OP.add)
                o3 = out_kndg.rearrange("p (k ndg) -> p k ndg", k=4).unsqueeze(2)
                nc.vector.tensor_tensor(out=o3, in0=t23[:, :, 0:1, :],
                                        in1=t23[:, :, 1:2, :], op=OP.add)

            def dtree2(src5, out_king):
                """Sum over d of [p,ki32,n,d,g] via tree-adds."""
                cur = src5
                nd = 16
                szs = {8: "t2048", 4: "t1024", 2: "t512"}
                while nd > 2:
                    nd //= 2
                    t = trees.tile([128, 32 * 2 * nd * 4], F16, tag=szs[nd],
                                   name=szs[nd])
                    t5 = t.rearrange("p (ki n d g) -> p ki n d g", ki=32, n=2,
                                     d=nd)
                    nc.vector.tensor_tensor(
                        out=t5, in0=cur[:, :, :, 0:nd, :],
                        in1=cur[:, :, :, nd:2 * nd, :], op=OP.add)
                    cur = t5
                o5 = kig2(out_king).unsqueeze(3)
                nc.vector.tensor_tensor(
                    out=o5, in0=cur[:, :, :, 0:1, :], in1=cur[:, :, :, 1:2, :],
                    op=OP.add)

            def split_tt(out5, in05, in15, op, cut):
                """One [p,ki32,n,d,g] TT, split: ki<cut on DVE, rest on Pool
                (concurrent engines, no serial coupling)."""
                if cut >= 32:
                    nc.vector.tensor_tensor(out=out5, in0=in05, in1=in15, op=op)
                    return
                nc.vector.tensor_tensor(
                    out=out5[:, 0:cut], in0=in05[:, 0:cut], in1=in15[:, 0:cut],
                    op=op)
                nc.gpsimd.tensor_tensor(
                    out=out5[:, cut:32], in0=in05[:, cut:32],
                    in1=in15[:, cut:32], op=op)

            def route_half(bl, h, cut_a, cut_b):
                T5 = TM5(T_tiles[bl])[:, h * 32:(h + 1) * 32]
                stM = stat_tiles[bl].rearrange(
                    "p (ki t n g) -> p ki t n g", ki=64, t=2, n=2)
                m1v = stM[:, h * 32:(h + 1) * 32, 0]   # [p,ki32,n,g]
                e2v = stM[:, h * 32:(h + 1) * 32, 1]

                # LayerNorm: var = E[t^2] - m1^2 ; rstd ; n2 = m1*rstd
                mm_ = sm.tile([128, 256], F16, tag="mm_", name="mm_")
                nc.vector.tensor_tensor(out=kig2(mm_), in0=m1v, in1=m1v,
                                        op=OP.mult)
                var = sm.tile([128, 256], F16, tag="var", name="var")
                nc.vector.tensor_tensor(out=kig2(var), in0=e2v, in1=kig2(mm_),
                                        op=OP.subtract)
                rstd = sm.tile([128, 256], F16, tag="rstd", name="rstd")
                nc.scalar.activation(rstd[:], var[:], AF.Sqrt, bias=eps5[:])
                nc.vector.reciprocal(rstd[:], rstd[:])
                n2 = sm.tile([128, 256], F16, tag="n2", name="n2")
                nc.vector.tensor_tensor(out=kig2(n2), in0=m1v, in1=kig2(rstd),
                                        op=OP.mult)

                # up = (T - m1) * rstd (optionally * gamma + beta)
                q = qp.tile([128, 4096], F16, tag="q", name="q")
                split_tt(HM5(q), T5, bc_kig2(rstd), OP.mult, cut_a)
                up = qp.tile([128, 4096], F16, tag="up", name="up")
                split_tt(HM5(up), HM5(q), bc_kig2(n2), OP.subtract, cut_a)
                if apply_gb:
                    up5 = HM5(up)
                    gb = (gam_sb[:].unsqueeze(1).unsqueeze(2).unsqueeze(4)
                          .broadcast_to((128, 32, 2, 16, 4)))
                    bb_ = (bet_sb[:].unsqueeze(1).unsqueeze(2).unsqueeze(4)
                           .broadcast_to((128, 32, 2, 16, 4)))
                    nc.vector.tensor_tensor(out=up5, in0=up5, in1=gb, op=OP.mult)
                    nc.vector.tensor_tensor(out=up5, in0=up5, in1=bb_, op=OP.add)

                # S = sum_i up ; dot_i = up_i . S
                S = sm.tile([128, 512], F16, tag="S", name="S")
                itree2(kin3(up), S)
                P = scr.tile([128, 4096], F16, tag="P", name="P")
                S_bc = (S.rearrange("p (k ndg) -> p k ndg", k=4).unsqueeze(2)
                        .broadcast_to((128, 4, 8, 128)))
                kc = cut_b // 8
                if kc >= 4:
                    nc.vector.tensor_tensor(out=kin3(P), in0=kin3(up),
                                            in1=S_bc, op=OP.mult)
                else:
                    nc.vector.tensor_tensor(
                        out=kin3(P)[:, 0:kc], in0=kin3(up)[:, 0:kc],
                        in1=S_bc[:, 0:kc], op=OP.mult)
                    nc.gpsimd.tensor_tensor(
                        out=kin3(P)[:, kc:4], in0=kin3(up)[:, kc:4],
                        in1=S_bc[:, kc:4], op=OP.mult)
                dot = sm.tile([128, 256], F16, tag="dot", name="dot")
                dtree2(HM5(P), dot)

                # rr_i = dot_i / max(||up_i||^2, 1e-8)
                rr = sm.tile([128, 256], F16, tag="rr", name="rr")
                if not apply_gb:
                    # 1/||up||^2 = (var+eps)/(16 var) = 1/16 + (eps/16)/var
                    ns_ = sm.tile([128, 256], F16, tag="ns_", name="ns_")
                    nc.vector.reciprocal(ns_[:], var[:])
                    nc.vector.tensor_scalar(
                        out=ns_[:], in0=ns_[:], scalar1=1e-5 / 16.0,
                        scalar2=1.0 / 16.0, op0=OP.mult, op1=OP.add)
                    nc.vector.tensor_tensor(out=rr[:], in0=dot[:], in1=ns_[:],
                                            op=OP.mult)
                else:
                    usq = scr.tile([128, 4096], F16, tag="P", name="usq")
                    nc.vector.tensor_tensor(out=usq[:], in0=up[:], in1=up[:],
                                            op=OP.mult)
                    nq = sm.tile([128, 256], F16, tag="nq", name="nq")
                    dtree2(HM5(usq), nq)
                    nc.vector.tensor_scalar_max(out=nq[:], in0=nq[:],
                                                scalar1=1e-8)
                    nc.vector.reciprocal(nq[:], nq[:])
                    nc.vector.tensor_tensor(out=rr[:], in0=dot[:], in1=nq[:],
                                            op=OP.mult)

                # softmax over i (shift-free: |rr| <= 8)
                es = sm.tile([128, 256], F16, tag="es", name="es")
                nc.scalar.activation(es[:], rr[:], AF.Exp)
                es4 = es.rearrange("p (k i n g) -> p k i n g", k=4, i=8, n=2)
                zt1 = trees.tile([128, 128], F16, tag="z4", name="z4")
                z14 = zt1.rearrange("p (k i n g) -> p k i n g", k=4, i=4, n=2)
                nc.vector.tensor_tensor(out=z14, in0=es4[:, :, 0:4],
                                        in1=es4[:, :, 4:8], op=OP.add)
                zt2 = trees.tile([128, 64], F16, tag="z2", name="z2")
                z24 = zt2.rearrange("p (k i n g) -> p k i n g", k=4, i=2, n=2)
                nc.vector.tensor_tensor(out=z24, in0=z14[:, :, 0:2],
                                        in1=z14[:, :, 2:4], op=OP.add)
                Z = sm.tile([128, 32], F16, tag="Z", name="Z")
                Z4 = Z.rearrange("p (k n g) -> p k n g", k=4, n=2).unsqueeze(2)
                nc.vector.tensor_tensor(out=Z4, in0=z24[:, :, 0:1],
                                        in1=z24[:, :, 1:2], op=OP.add)
                nc.vector.reciprocal(Z[:], Z[:])
                sc = sm.tile([128, 256], F16, tag="sc", name="sc")
                Zb = (Z.rearrange("p (k n g) -> p k n g", k=4, n=2).unsqueeze(2)
                      .broadcast_to((128, 4, 8, 2, 4)))
                sc4 = sc.rearrange("p (k i n g) -> p k i n g", k=4, i=8, n=2)
                nc.vector.tensor_tensor(out=sc4, in0=es4, in1=Zb, op=OP.mult)

                # s = sum_i score_i up_i ; squash over g
                P2 = scr.tile([128, 4096], F16, tag="P", name="P2")
                split_tt(HM5(P2), HM5(up), bc_kig2(sc), OP.mult, cut_b)
                s = sm.tile([128, 512], F16, tag="s", name="s")
                itree2(kin3(P2), s)
                ssq = sm.tile([128, 512], F16, tag="ssq", name="ssq")
                nc.vector.tensor_tensor(out=ssq[:], in0=s[:], in1=s[:], op=OP.mult)
                s4 = ssq.rearrange("p (knd g) -> p knd g", knd=128)
                gt = trees.tile([128, 256], F16, tag="g2", name="g2")
                gt4 = gt.rearrange("p (knd g) -> p knd g", knd=128)
                nc.vector.tensor_tensor(out=gt4, in0=s4[:, :, 0:2],
                                        in1=s4[:, :, 2:4], op=OP.add)
                nsq = sm.tile([128, 128], F16, tag="nsq", name="nsq")
                nsq4 = nsq[:].unsqueeze(2)
                nc.vector.tensor_tensor(out=nsq4, in0=gt4[:, :, 0:1],
                                        in1=gt4[:, :, 1:2], op=OP.add)
                sr = sm.tile([128, 128], F16, tag="sr", name="sr")
                nc.scalar.activation(sr[:], nsq[:], AF.Sqrt, bias=eps16[:])
                d1 = sm.tile([128, 128], F16, tag="d1", name="d1")
                nc.vector.scalar_tensor_tensor(
                    out=d1[:], in0=nsq[:], scalar=1.0, in1=sr[:],
                    op0=OP.add, op1=OP.mult)
                nc.vector.reciprocal(d1[:], d1[:])
                f = sm.tile([128, 128], F16, tag="f", name="f")
                nc.vector.tensor_tensor(out=f[:], in0=nsq[:], in1=d1[:], op=OP.mult)
                v = vp.tile([128, 512], F16, tag="v", name="v")
                v4 = v.rearrange("p (knd g) -> p knd g", knd=128)
                fb = f[:].unsqueeze(2).broadcast_to((128, 128, 4))
                s44 = s.rearrange("p (knd g) -> p knd g", knd=128)
                nc.vector.tensor_tensor(out=v4, in0=s44, in1=fb, op=OP.mult)
                u_ = bl * 1024 + h * 512
                nc.sync.dma_start(out=outd.ap()[:, u_:u_ + 512], in_=v[:])

            for bl in range(2):
                for chh in range(2):
                    conv_pass(bl, chh)
                    transS_half(bl, chh)
                    transT_half(bl, chh)
            route_half(0, 0, cut_a=32, cut_b=32)
            route_half(0, 1, cut_a=32, cut_b=24)
            route_half(1, 0, cut_a=24, cut_b=24)
            route_half(1, 1, cut_a=24, cut_b=24)

    _split_sync_waits(nc)
    return nc


def _pack_weights(conv_w):
    w = np.asarray(conv_w, np.float32)
    wt = np.stack(
        [np.roll(np.rot90(w, k=r, axes=(3, 4)), r, axis=2) for r in range(4)], axis=1
    )
    W512 = np.ascontiguousarray(wt.reshape(512, 64, 3, 3), dtype=np.float32)
    packs = []
    for pi in range(4):
        Wc = W512[128 * pi: 128 * pi + 128]  # 2 nouts' channels (n,d,g)
        w_pack = np.zeros((128, 6, 128), np.float32)
        for kx in range(3):
            w_pack[0:64, kx] = Wc[:, :, 0, kx].T
            w_pack[64:128, kx] = Wc[:, :, 1, kx].T
            w_pack[0:64, 3 + kx] = Wc[:, :, 2, kx].T
        packs.append(np.ascontiguousarray(
            w_pack.reshape(128, 768), dtype=np.float16))
    return packs


def _pack_caps(capsules):
    """[4,8,16,4,32,32] f32 -> [32,128,1164] f16 (padded image + row-shifted
    copy per [128]-partition tile)."""
    x = np.asarray(capsules, np.float32).reshape(32, 64, 32, 32)
    pad = np.zeros((32, 64, 34, 34), np.float16)
    pad[:, :, 1:33, 1:33] = x.astype(np.float16)
    A = pad.reshape(32, 64, 1156)
    buf = np.zeros((32, 128, 1164), np.float16)
    buf[:, 0:64, 0:1156] = A
    buf[:, 64:128, 0:1122] = A[:, :, 34:1156]
    return buf


_CACHE = {}


def kernel(capsules, conv_w, conv_b, ln_gamma, ln_beta):
    conv_b = np.asarray(conv_b, np.float32)
    ln_gamma = np.asarray(ln_gamma, np.float32)
    ln_beta = np.asarray(ln_beta, np.float32)
    apply_bias = bool(np.any(conv_b))
    apply_gb = bool(np.any(ln_gamma != 1.0) or np.any(ln_beta != 0.0))

    key = (apply_bias, apply_gb)
    if key not in _CACHE:
        _CACHE[key] = build_program(apply_bias=apply_bias, apply_gb=apply_gb)
    nc = _CACHE[key]

    capsd = _pack_caps(capsules)
    packs = _pack_weights(conv_w)
    identity = np.eye(128, dtype=np.float16)
    mmu = np.zeros((128, 16), np.float16)
    for ch in range(128):
        nn_, gg = ch // 64, ch % 4
        for t in range(2):
            mmu[ch, t * 8 + nn_ * 4 + gg] = 1.0 / 16.0
    in_maps = []
    for c in range(8):
        beta_ = c // 4   # batch-pair
        pi = c % 4       # nout-pair
        m = {"capsd": np.ascontiguousarray(capsd[16 * beta_: 16 * beta_ + 16]),
             "w": packs[pi], "ident": identity, "mmu": mmu}
        if apply_bias:
            # channel order (n,d,g): n*64 + d*4 + g
            b_loc = np.repeat(conv_b[32 * pi: 32 * pi + 32], 4).astype(np.float16)
            m["cb"] = np.ascontiguousarray(b_loc.reshape(1, 128))
        if apply_gb:
            m["gam"] = np.ascontiguousarray(ln_gamma.reshape(1, 16), dtype=np.float16)
            m["bet"] = np.ascontiguousarray(ln_beta.reshape(1, 16), dtype=np.float16)
        in_maps.append(m)

    res = run_bass_kernel_spmd(nc, in_maps, core_ids=list(range(8)), trace=False)
    # per-core out: [128, 2048] f16 = (p, bl, n, k, d, g); position = k*128+p
    out = np.zeros((4, 8, 16, 4, 32, 32), np.float32)
    for c in range(8):
        beta_, pi = c // 4, c % 4
        r = np.asarray(res.results[c]["outd"], np.float32).reshape(128, 2, 8, 2, 16, 4)
        for bl in range(2):
            for n in range(2):
                out[2 * beta_ + bl, 2 * pi + n] = (
                    r[:, bl, :, n].transpose(2, 3, 1, 0).reshape(16, 4, 32, 32))
    return out
